# revision 18
# baseline (speedup 1.0000x reference)
"""Trainium2 Bass kernel v3 for nn_BestEllipseLoss_5720896438361.

v3 over v2: fields moved to ACT engine (Relu/Sign with constant bias from a
bf16 normalized image), moment matmuls regrouped to 16x(N=512, M=96),
interval-sum reduce split into a bf16 halving fold + smaller f32-out reduce,
P2 memset shrunk to segment boundary columns, pure-f32 output-side dfield.

kernel(output, target): full [512,128,128] f32 inputs -> scalar f32 loss.
Shards batch across 8 NeuronCores (64 samples each), one SPMD Bass kernel.
"""
import sys
if "/opt/trn_rl_repo" not in sys.path:
    sys.path.insert(0, "/opt/trn_rl_repo")

import numpy as np

import concourse.bass as bass
import concourse.bacc as bacc
import concourse.tile as tile
import concourse.mybir as mybir
import concourse.bass_isa as bass_isa
import bass_rust

F32 = mybir.dt.float32
BF16 = mybir.dt.bfloat16
I32 = mybir.dt.int32
I16 = mybir.dt.int16
Alu = mybir.AluOpType
Act = mybir.ActivationFunctionType
AX = mybir.AxisListType

EPS = np.float32(1e-8)
LEVELS = [np.float32(0.3), np.float32(0.4), np.float32(0.5), np.float32(0.6), np.float32(0.7)]
NL = 5
H = 128
W = 128
NPIX = float(H * W)

_x = np.arange(W, dtype=np.float64)
_y = np.arange(H, dtype=np.float64)
# basis order j: {1, y, y^2, x, x*y, x^2}
C_B = np.array([
    H * W, W * _y.sum(), W * (_y ** 2).sum(),
    H * _x.sum(), _x.sum() * _y.sum(), H * (_x ** 2).sum(),
], dtype=np.float64).astype(np.float32)

# marker-scatter geometry
SEG = 130            # per-sample segment width in D / P2
SPW = 13             # samples per scatter window (SPW*SEG <= 2046)
NW = 5               # windows
NSLOT = SPW * NW     # 65 slots (64 real + 1 dummy)
DW = NSLOT * SEG     # 8450
WELEM = SPW * SEG    # 1690 elements per scatter call

# moment matmul grouping: x-chunk 8, basis j=12 (hi/lo), M=96, N=512
XC = 8               # x positions per group
NG = W // XC         # 16 groups
MROW = XC * 12       # 96 stationary rows


def _scan3d(eng, out, data0, data1, initial, op0, op1):
    """tensor_tensor_scan with multi-free-dim APs (recurrence chains across
    slices; data0 mask handles segment resets)."""
    return eng.add_instruction(
        mybir.InstTensorScalarPtr(
            name=eng.bass.get_next_instruction_name(),
            is_tensor_tensor_scan=True,
            is_scalar_tensor_tensor=True,
            op0=op0,
            op1=op1,
            ins=[
                eng.lower_ap(data0),
                eng.lower_ap_or_imm(initial),
                eng.lower_ap(data1),
            ],
            outs=[eng.lower_ap(out)],
        )
    )


def _ovl(ap, nslot, seg, stride):
    """Overlapping windowed view [(p), nslot, seg] with window stride."""
    v = ap.copy()
    v.ap = bass_rust.VecI64Pair([list(ap.ap[0]), [stride, nslot], [1, seg]])
    return v


def emit(nc, tc, NS=64):
    F = NS * W
    SL2 = NL * NSLOT      # 325, (l-major, 65-slot) layout

    t_in = nc.dram_tensor("t", [H, NS, W], F32, kind="ExternalInput")
    o_in = nc.dram_tensor("o", [H, NS, W], F32, kind="ExternalInput")
    loss_out = nc.dram_tensor("loss", [NS, 1], F32, kind="ExternalOutput")

    with tc.tile_pool(name="big", bufs=1) as big, \
         tc.tile_pool(name="dpool", bufs=2) as dpool, \
         tc.tile_pool(name="fg", bufs=1) as fg, \
         tc.tile_pool(name="med", bufs=1) as med, \
         tc.tile_pool(name="sml", bufs=1) as sml, \
         tc.tile_pool(name="ps", bufs=2, space="PSUM") as ps, \
         tc.tile_pool(name="ps1", bufs=1, space="PSUM") as ps1:

        # ================= constants =================
        yi = sml.tile([128, 1], I32)
        nc.gpsimd.iota(yi[:], pattern=[[0, 1]], base=0, channel_multiplier=1)
        yv = sml.tile([128, 1], F32)
        nc.vector.tensor_copy(yv[:], yi[:])
        y2v = sml.tile([128, 1], F32)
        nc.vector.tensor_tensor(out=y2v[:], in0=yv[:], in1=yv[:], op=Alu.mult)

        ei = med.tile([128, 128], I32, tag="scrA")
        nc.gpsimd.iota(ei[:], pattern=[[1, 128]], base=0, channel_multiplier=-1)
        eif = med.tile([128, 128], F32, tag="scrB")
        nc.vector.tensor_copy(eif[:], ei[:])
        eye128 = med.tile([128, 128], F32)
        nc.vector.tensor_scalar(eye128[:], eif[:], 0.0, None, Alu.is_equal)

        onescol = sml.tile([128, 1], F32)
        nc.gpsimd.memset(onescol[:], 1.0)
        onescolb = sml.tile([128, 1], BF16)
        nc.gpsimd.memset(onescolb[:], 1.0)

        lvl_bias = []
        for l in range(NL):
            b = sml.tile([128, 1], F32, name=f"lvlb{l}")
            nc.gpsimd.memset(b[:], -float(LEVELS[l] - np.float32(0.5)))
            lvl_bias.append(b)

        # SEL_i [96, 6] fold matrices: pick q = 12*i + j (hi) and 12*i+6+j (lo)
        di = med.tile([MROW, 6], I32, tag="scrC")
        nc.gpsimd.iota(di[:], pattern=[[-1, 6]], base=0, channel_multiplier=1)
        df = med.tile([MROW, 6], F32, tag="scrD")
        nc.vector.tensor_copy(df[:], di[:])
        SELS = []
        for i in range(XC):
            s1 = med.tile([MROW, 6], F32, name=f"sel{i}")
            nc.vector.tensor_scalar(s1[:], df[:], float(12 * i), None, Alu.is_equal)
            s2 = med.tile([MROW, 6], F32, name=f"sel2_{i}", tag="scrE")
            nc.vector.tensor_scalar(s2[:], df[:], float(12 * i + 6), None, Alu.is_equal)
            nc.vector.tensor_tensor(out=s1[:], in0=s1[:], in1=s2[:], op=Alu.add)
            SELS.append(s1)

        # ---- moment lhsT table [128, NG*MROW] bf16 (hi/lo split basis) ----
        mast = med.tile([128, 768], F32, tag="scrA")
        nc.gpsimd.memset(mast[:], 1.0)
        xri = sml.tile([1, W], I32)
        nc.gpsimd.iota(xri[:], pattern=[[1, W]], base=0, channel_multiplier=0)
        xrf = sml.tile([1, W], F32)
        nc.vector.tensor_copy(xrf[:], xri[:])
        x2rf = sml.tile([1, W], F32)
        nc.vector.tensor_tensor(out=x2rf[:], in0=xrf[:], in1=xrf[:], op=Alu.mult)
        xfull = med.tile([128, W], F32, tag="scrB")
        nc.gpsimd.partition_broadcast(xfull[:], xrf[:], channels=128)
        x2full = med.tile([128, W], F32, tag="scrF")
        nc.gpsimd.partition_broadcast(x2full[:], x2rf[:], channels=128)
        mv = mast[:].rearrange("p (g t j) -> p g t j", g=NG, t=XC)
        xfv = xfull[:].rearrange("p (g t) -> p g t", g=NG)
        x2fv = x2full[:].rearrange("p (g t) -> p g t", g=NG)
        nc.vector.tensor_copy(mv[:, :, :, 3:4], xfv.to_broadcast((128, NG, XC, 1)))
        nc.vector.tensor_copy(mv[:, :, :, 4:5], xfv.to_broadcast((128, NG, XC, 1)))
        nc.vector.tensor_copy(mv[:, :, :, 5:6], x2fv.to_broadcast((128, NG, XC, 1)))
        mgt = mast[:].rearrange("p (gt j) -> p gt j", j=6)
        nc.vector.tensor_scalar(mgt[:, :, 1:2], mgt[:, :, 1:2], yv[:], None, Alu.mult)
        nc.vector.tensor_scalar(mgt[:, :, 4:5], mgt[:, :, 4:5], yv[:], None, Alu.mult)
        nc.vector.tensor_scalar(mgt[:, :, 2:3], mgt[:, :, 2:3], y2v[:], None, Alu.mult)
        hi24 = med.tile([128, 768], BF16, tag="scrG")
        nc.vector.tensor_copy(hi24[:], mast[:])
        lo24 = mast
        nc.vector.tensor_tensor(out=lo24[:], in0=mast[:], in1=hi24[:], op=Alu.subtract)
        table = med.tile([128, NG * MROW], BF16)
        tvv = table[:].rearrange("p (g t j) -> p g t j", g=NG, t=XC)
        nc.vector.tensor_copy(tvv[:, :, :, 0:6], hi24[:].rearrange("p (g t j) -> p g t j", g=NG, t=XC))
        nc.vector.tensor_copy(tvv[:, :, :, 6:12], lo24[:].rearrange("p (g t j) -> p g t j", g=NG, t=XC))

        # lvl rows [NS, NL]
        lr_i = sml.tile([NS, NL], I32)
        nc.gpsimd.iota(lr_i[:], pattern=[[1, NL]], base=0, channel_multiplier=0)

        # marker bases: LB1 = 130*sl + 1 over (l, w, sl)
        lb_i = med.tile([128, SL2], I32, tag="nint")
        nc.gpsimd.iota(lb_i[:].rearrange("p (l w s) -> p l w s", l=NL, w=NW),
                       pattern=[[0, NL], [0, NW], [SEG, SPW]], base=1,
                       channel_multiplier=0)
        LB1 = med.tile([128, SL2], F32)
        nc.vector.tensor_copy(LB1[:], lb_i[:])
        # rank-2 lhsT for arg/mrow matmuls: Y3 = [y^2; y; 1], Y2 = [1; -y]
        y3c = med.tile([128, 3], F32, tag="scrD2")
        nc.vector.tensor_copy(y3c[:, 0:1], y2v[:])
        nc.vector.tensor_copy(y3c[:, 1:2], yv[:])
        nc.vector.tensor_scalar(y3c[:, 2:3], yv[:], 0.0, 1.0, Alu.mult, Alu.add)
        psy = ps1.tile([3, 128], F32, tag="psmisc")
        nc.tensor.transpose(psy[:], y3c[:], eye128[:, :])
        Y3 = med.tile([3, 128], F32)
        nc.vector.tensor_copy(Y3[:], psy[:])
        y2c = med.tile([128, 2], F32, tag="scrD3")
        nc.vector.tensor_scalar(y2c[:, 0:1], yv[:], 0.0, 1.0, Alu.mult, Alu.add)
        nc.vector.tensor_scalar(y2c[:, 1:2], yv[:], -1.0, None, Alu.mult)
        psy2 = ps1.tile([2, 128], F32, tag="psmisc")
        nc.tensor.transpose(psy2[:], y2c[:], eye128[:, :])
        Y2 = med.tile([2, 128], F32)
        nc.vector.tensor_copy(Y2[:], psy2[:])

        # scatter data row: (-1, +1) x SPW in bf16
        d26i = sml.tile([128, SPW * 2], I32)
        nc.gpsimd.iota(d26i[:], pattern=[[0, SPW], [1, 2]], base=0, channel_multiplier=0)
        d26f = sml.tile([128, SPW * 2], F32)
        nc.vector.tensor_copy(d26f[:], d26i[:])
        data26 = sml.tile([128, SPW * 2], BF16)
        nc.vector.tensor_scalar(data26[:], d26f[:], 2.0, -1.0, Alu.mult, Alu.add)

        # ================= loads =================
        hp = tc.high_priority(offset=100000)
        hp.__enter__()
        # 4-queue split load: each engine's DGE processes its descriptor batch
        # concurrently, ~4x the single-queue descriptor rate
        vt = big.tile([128, F], F32, tag="bigA")
        tv_flat = t_in[:, :, :].rearrange("y s x -> y (s x)")
        for i in range(4):
            eng = (nc.sync, nc.scalar)[i % 2]
            eng.dma_start(vt[:, i * (F // 4):(i + 1) * (F // 4)],
                          tv_flat[:, i * (F // 4):(i + 1) * (F // 4)])
        vo = big.tile([128, F], F32, tag="bigO", name="vo")
        ov_flat = o_in[:, :, :].rearrange("y s x -> y (s x)")
        for i in range(2):
            eng = (nc.sync, nc.scalar)[i % 2]
            eng.dma_start(vo[:, i * (F // 2):(i + 1) * (F // 2)],
                          ov_flat[:, i * (F // 2):(i + 1) * (F // 2)])

        # ================= min/max + normalize (target) =================
        # per-partition max / negated min land side by side in mmT; one PE
        # transpose + free-dim reduce replaces gpsimd partition_all_reduce
        mmT = med.tile([128, 2 * NS], F32, name="mmT", tag="mxp")
        NH2 = NS // 2
        for hi2 in range(2):
            hv = vt[:, hi2 * NH2 * W:(hi2 + 1) * NH2 * W].rearrange("p (s x) -> p s x", s=NH2)
            nc.vector.tensor_reduce(mmT[:, hi2 * NH2:(hi2 + 1) * NH2], hv, AX.X, Alu.max)
            nc.vector.tensor_reduce(mmT[:, NS + hi2 * NH2:NS + (hi2 + 1) * NH2], hv, AX.X, Alu.min)
        nc.vector.tensor_scalar(mmT[:, NS:2 * NS], mmT[:, NS:2 * NS], -1.0, None, Alu.mult)
        # separate transposes rebase both halves to partition 0
        psmxT = ps1.tile([NS, 128], F32, tag="psarg")
        nc.tensor.transpose(psmxT[:], mmT[:, 0:NS], eye128[:, :])
        mxcT = med.tile([NS, 1], F32, name="mxcT")
        nc.vector.tensor_reduce(mxcT[:], psmxT[:], AX.X, Alu.max)
        psmnT = ps1.tile([NS, 128], F32, tag="psarg")
        nc.tensor.transpose(psmnT[:], mmT[:, NS:2 * NS], eye128[:, :])
        negmnT = med.tile([NS, 1], F32, name="negmnT")
        nc.vector.tensor_reduce(negmnT[:], psmnT[:], AX.X, Alu.max)
        rngcT = med.tile([NS, 1], F32, name="rngcT")
        nc.vector.tensor_tensor(out=rngcT[:], in0=mxcT[:], in1=negmnT[:], op=Alu.add)
        nc.vector.tensor_scalar(rngcT[:], rngcT[:], float(EPS), None, Alu.add)
        rbcT = med.tile([NS, 1], F32, name="rbcT")
        nc.vector.reciprocal(rbcT[:], rngcT[:])
        shcT = med.tile([NS, 1], F32, name="shcT")
        nc.vector.tensor_scalar(shcT[:], rngcT[:], 0.5, None, Alu.mult)
        nc.vector.tensor_tensor(out=shcT[:], in0=shcT[:], in1=negmnT[:], op=Alu.subtract)
        # rowify (shift, rb) -> [128, NS] broadcast rows
        def rowify(colap, bcast_out, nm):
            pr = ps1.tile([1, NS], F32, tag="psmisc", name=f"psr{nm}")
            nc.tensor.transpose(pr[:], colap, eye128[0:NS, 0:NS])
            r = med.tile([1, NS], F32, name=f"row{nm}")
            nc.vector.tensor_copy(r[:], pr[:])
            nc.gpsimd.partition_broadcast(bcast_out, r[:], channels=128)
            return r

        shiftT = med.tile([128, NS], F32)
        rowify(shcT[:], shiftT[:], "shT")
        rbT = med.tile([128, NS], F32)
        rowify(rbcT[:], rbT[:], "rbT")

        for hi2 in range(2):
            upv = vt[:, hi2 * NH2 * W:(hi2 + 1) * NH2 * W].rearrange("p (s x) -> p s x", s=NH2)
            sh = shiftT[:, hi2 * NH2:(hi2 + 1) * NH2]
            rb = rbT[:, hi2 * NH2:(hi2 + 1) * NH2]
            nc.vector.tensor_tensor(out=upv, in0=upv,
                                    in1=sh.to_broadcast((128, NH2, W)), op=Alu.subtract)
            nc.vector.tensor_tensor(out=upv, in0=upv,
                                    in1=rb.to_broadcast((128, NH2, W)), op=Alu.mult)
        # bf16 copy of normalized centered image (on ACT engine); lives in the
        # dpool rotation (dead before the second scatter tile needs the slot)
        upb = dpool.tile([128, DW], BF16, tag="dtile", name="upb")
        nc.scalar.activation(upb[:, 0:F], vt[:, 0:F], Act.Copy, bias=0.0, scale=1.0)

        # ============ segmented prefix scan -> P2 (bf16, windowed) ============
        # state = (rowmask * state) + u' resets at each sample's x=0; output
        # lands at window slot x+1, so slot k = sum_{x<k} u' per sample.
        mi_ = sml.tile([128, W], I32, name="mi_")
        nc.gpsimd.iota(mi_[:], pattern=[[1, W]], base=0, channel_multiplier=0)
        mf_ = sml.tile([128, W], F32, name="mf_")
        nc.vector.tensor_copy(mf_[:], mi_[:])
        nc.vector.tensor_scalar(mf_[:], mf_[:], 0.5, None, Alu.is_ge)
        P2 = big.tile([128, DW], BF16, name="P2", tag="bigP")
        p2segs = P2[:].rearrange("p (s c) -> p s c", s=NSLOT)
        nc.gpsimd.memset(p2segs[:, :, 0:1], 0.0)
        nc.gpsimd.memset(p2segs[:, :, 129:130], 0.0)
        _scan3d(nc.vector,
                _ovl(P2[:, 1:], NS, W, SEG),
                _ovl(mf_[:, :], NS, W, 0),
                vt[:, 0:F].rearrange("p (s x) -> p s x", s=NS),
                0.0, Alu.mult, Alu.add)

        # itot from P2 column 128: itot(s) = sum_y P'[y, s, 128]
        p2col = P2[:].rearrange("p (s c) -> p s c", s=NSLOT)[:, :, 128]
        psb = ps1.tile([1, NSLOT], F32, tag="psb")
        nc.tensor.matmul(psb[:], onescolb[:], p2col, start=True, stop=True)
        brow = sml.tile([1, NSLOT], F32)
        nc.vector.tensor_copy(brow[:], psb[:])
        itot = sml.tile([1, NS], F32)
        nc.vector.tensor_scalar(itot[:], brow[:, 0:NS], 0.5 * NPIX, float(EPS), Alu.add, Alu.add)
        itr = sml.tile([1, NS], F32)
        nc.vector.reciprocal(itr[:], itot[:])

        # ================= fields + moments (target) =================
        SA = med.tile([NS, 72], F32)

        def moments(field_tile, fl, out_tile, col0):
            psm = ps.tile([MROW, NS * XC], F32, name=f"psm{fl}", tag="psmom")
            fv = field_tile[:, 0:F].rearrange("p (s x) -> p s x", s=NS)
            for g in range(NG):
                nc.tensor.matmul(psm[:], table[:, g * MROW:(g + 1) * MROW],
                                 fv[:, :, XC * g:XC * g + XC],
                                 start=(g == 0), stop=(g == NG - 1))
            S96 = med.tile([MROW, NS * XC], F32, name=f"s96_{fl}", tag="s96")
            nc.vector.tensor_copy(S96[:], psm[:])
            pT = ps.tile([NS, 6], F32, name=f"pT{fl}", tag="ps6")
            sv = S96[:].rearrange("q (s t) -> q s t", s=NS)
            for i in range(XC):
                nc.tensor.matmul(pT[:], sv[:, :, i], SELS[i][:],
                                 start=(i == 0), stop=(i == XC - 1))
            nc.vector.tensor_copy(out_tile[:, col0:col0 + 6], pT[:])

        for l in range(NL):
            lvlp = float(LEVELS[l] - np.float32(0.5))
            f_l = fg.tile([128, F], BF16, name=f"f{l}", tag="ffield")
            nc.vector.tensor_scalar(f_l[:], upb[:, 0:F], lvlp, lvlp, Alu.max, Alu.subtract)
            g_l = fg.tile([128, F], BF16, name=f"g{l}", tag="gfield")
            nc.scalar.activation(g_l[:], vt[:, 0:F], Act.Sign, bias=lvl_bias[l][:], scale=1.0)
            moments(f_l, l, SA, l * 6)
            moments(g_l, 5 + l, SA, 30 + l * 6)

        hp.__exit__(None, None, None)

        # minmaxO per-half (slots into field-phase DVE gaps)
        mmO = med.tile([128, 2 * NS], F32, name="mmO", tag="mxp")
        for ho in range(2):
            hv = vo[:, ho * (F // 2):(ho + 1) * (F // 2)].rearrange(
                "p (s x) -> p s x", s=NS // 2)
            nc.vector.tensor_reduce(mmO[:, ho * (NS // 2):(ho + 1) * (NS // 2)], hv, AX.X, Alu.max)
            nc.vector.tensor_reduce(mmO[:, NS + ho * (NS // 2):NS + (ho + 1) * (NS // 2)], hv, AX.X, Alu.min)
        nc.vector.tensor_scalar(mmO[:, NS:2 * NS], mmO[:, NS:2 * NS], -1.0, None, Alu.mult)
        psmxO = ps1.tile([NS, 128], F32, tag="psarg")
        nc.tensor.transpose(psmxO[:], mmO[:, 0:NS], eye128[:, :])
        mxcO = med.tile([NS, 1], F32, name="mxcO")
        nc.vector.tensor_reduce(mxcO[:], psmxO[:], AX.X, Alu.max)
        psmnO = ps1.tile([NS, 128], F32, tag="psarg")
        nc.tensor.transpose(psmnO[:], mmO[:, NS:2 * NS], eye128[:, :])
        negmnO = med.tile([NS, 1], F32, name="negmnO")
        nc.vector.tensor_reduce(negmnO[:], psmnO[:], AX.X, Alu.max)
        mncO = med.tile([NS, 1], F32, name="mncO")
        nc.vector.tensor_scalar(mncO[:], negmnO[:], -1.0, None, Alu.mult)
        rngcO = med.tile([NS, 1], F32, name="rngcO")
        nc.vector.tensor_tensor(out=rngcO[:], in0=mxcO[:], in1=negmnO[:], op=Alu.add)
        mnbO = med.tile([128, NS], F32, name="mnbO")
        rowify(mncO[:], mnbO[:], "mnO")
        rngbO = med.tile([128, NS], F32, name="rngbO")
        rowify(rngcO[:], rngbO[:], "rngO")

        # ================= target params (sample layout [NS, NL]) =================
        def FA(j):
            return SA[:, j:j + 30:6]

        def GA(j):
            return SA[:, 30 + j:30 + j + 30:6]

        cbt = med.tile([NS, 30], F32, name="cbt")
        hlvl = med.tile([NS, 30], F32, name="hlvl")
        for j in range(6):
            nc.gpsimd.memset(cbt[:, j:30:6], float(C_B[j]))
        for l in range(NL):
            nc.gpsimd.memset(hlvl[:, l * 6:(l + 1) * 6], 0.5 * float(LEVELS[l]))
        Wall = med.tile([NS, 30], F32, name="Wall")
        nc.vector.tensor_tensor(out=Wall[:], in0=SA[:, 30:60], in1=cbt[:], op=Alu.add)
        nc.vector.tensor_tensor(out=Wall[:], in0=Wall[:], in1=hlvl[:], op=Alu.mult)
        nc.vector.tensor_tensor(out=Wall[:], in0=Wall[:], in1=SA[:, 0:30], op=Alu.add)
        Wm = {j: Wall[:, j:30:6] for j in range(6)}

        def fit_params(Wd, tagp, n_l, want_roots, refine=True):
            def nt(nm):
                return med.tile([NS, n_l], F32, name=tagp + nm)
            m00_ = nt("m00")
            nc.vector.tensor_scalar(m00_[:], Wd[0], float(EPS), None, Alu.add)
            im_ = nt("im")
            nc.vector.reciprocal(im_[:], m00_[:])
            cx_ = nt("cx"); cy_ = nt("cy"); tz = nt("tz")
            nc.vector.tensor_tensor(out=cx_[:], in0=Wd[3], in1=im_[:], op=Alu.mult)
            nc.vector.tensor_tensor(out=cy_[:], in0=Wd[1], in1=im_[:], op=Alu.mult)
            mu20_ = nt("mu20"); mu02_ = nt("mu02"); mu11_ = nt("mu11")
            nc.vector.tensor_tensor(out=mu20_[:], in0=Wd[5], in1=im_[:], op=Alu.mult)
            nc.vector.tensor_tensor(out=tz[:], in0=cx_[:], in1=cx_[:], op=Alu.mult)
            nc.vector.tensor_tensor(out=mu20_[:], in0=mu20_[:], in1=tz[:], op=Alu.subtract)
            nc.vector.tensor_tensor(out=mu02_[:], in0=Wd[2], in1=im_[:], op=Alu.mult)
            nc.vector.tensor_tensor(out=tz[:], in0=cy_[:], in1=cy_[:], op=Alu.mult)
            nc.vector.tensor_tensor(out=mu02_[:], in0=mu02_[:], in1=tz[:], op=Alu.subtract)
            nc.vector.tensor_tensor(out=mu11_[:], in0=Wd[4], in1=im_[:], op=Alu.mult)
            nc.vector.tensor_tensor(out=tz[:], in0=cx_[:], in1=cy_[:], op=Alu.mult)
            nc.vector.tensor_tensor(out=mu11_[:], in0=mu11_[:], in1=tz[:], op=Alu.subtract)
            dmu_ = nt("dmu"); smu_ = nt("smu"); cc_ = nt("cc")
            nc.vector.tensor_tensor(out=dmu_[:], in0=mu20_[:], in1=mu02_[:], op=Alu.subtract)
            nc.vector.tensor_tensor(out=smu_[:], in0=mu20_[:], in1=mu02_[:], op=Alu.add)
            nc.vector.tensor_tensor(out=cc_[:], in0=dmu_[:], in1=dmu_[:], op=Alu.mult)
            nc.vector.tensor_tensor(out=tz[:], in0=mu11_[:], in1=mu11_[:], op=Alu.mult)
            nc.vector.tensor_scalar(tz[:], tz[:], 4.0, None, Alu.mult)
            nc.vector.tensor_tensor(out=cc_[:], in0=cc_[:], in1=tz[:], op=Alu.add)
            com_ = nt("com")
            nc.scalar.sqrt(com_[:], cc_[:])
            gd = nt("gd"); rc = nt("rc")
            if refine:
                nc.vector.tensor_scalar(gd[:], com_[:], 1e-30, None, Alu.max)
                nc.vector.reciprocal(rc[:], gd[:])
                nc.vector.tensor_tensor(out=rc[:], in0=cc_[:], in1=rc[:], op=Alu.mult)
                nc.vector.tensor_tensor(out=com_[:], in0=com_[:], in1=rc[:], op=Alu.add)
                nc.vector.tensor_scalar(com_[:], com_[:], 0.5, None, Alu.mult)
            a2_ = nt("a2"); b2_ = nt("b2")
            nc.vector.tensor_tensor(out=a2_[:], in0=smu_[:], in1=com_[:], op=Alu.add)
            nc.vector.tensor_scalar(a2_[:], a2_[:], 2.0, float(EPS), Alu.mult, Alu.max)
            nc.vector.tensor_tensor(out=b2_[:], in0=smu_[:], in1=com_[:], op=Alu.subtract)
            nc.vector.tensor_scalar(b2_[:], b2_[:], 2.0, float(EPS), Alu.mult, Alu.max)
            a_ = nt("a"); b_ = nt("b")
            nc.scalar.sqrt(a_[:], a2_[:])
            nc.scalar.sqrt(b_[:], b2_[:])
            if refine:
                nc.vector.tensor_scalar(gd[:], a_[:], 1e-30, None, Alu.max)
                nc.vector.reciprocal(rc[:], gd[:])
                nc.vector.tensor_tensor(out=rc[:], in0=a2_[:], in1=rc[:], op=Alu.mult)
                nc.vector.tensor_tensor(out=a_[:], in0=a_[:], in1=rc[:], op=Alu.add)
                nc.vector.tensor_scalar(a_[:], a_[:], 0.5, None, Alu.mult)
                nc.vector.tensor_scalar(gd[:], b_[:], 1e-30, None, Alu.max)
                nc.vector.reciprocal(rc[:], gd[:])
                nc.vector.tensor_tensor(out=rc[:], in0=b2_[:], in1=rc[:], op=Alu.mult)
                nc.vector.tensor_tensor(out=b_[:], in0=b_[:], in1=rc[:], op=Alu.add)
                nc.vector.tensor_scalar(b_[:], b_[:], 0.5, None, Alu.mult)
            cg = nt("cg"); ic = nt("ic")
            nc.vector.tensor_scalar(cg[:], com_[:], 1e-30, None, Alu.max)
            nc.vector.reciprocal(ic[:], cg[:])
            cphi_ = nt("cphi"); sphi_ = nt("sphi")
            nc.vector.tensor_tensor(out=cphi_[:], in0=dmu_[:], in1=ic[:], op=Alu.mult)
            nc.vector.tensor_scalar(cphi_[:], cphi_[:], -1.0, 1.0, Alu.max, Alu.min)
            nc.vector.tensor_tensor(out=sphi_[:], in0=mu11_[:], in1=ic[:], op=Alu.mult)
            cth_ = nt("cth"); sth_ = nt("sth"); sg_ = nt("sg")
            nc.vector.tensor_scalar(cth_[:], cphi_[:], 1.0, 0.5, Alu.add, Alu.mult)
            nc.scalar.sqrt(cth_[:], cth_[:])
            nc.vector.tensor_scalar(sth_[:], cphi_[:], -1.0, None, Alu.mult)
            nc.vector.tensor_scalar(sth_[:], sth_[:], 1.0, 0.5, Alu.add, Alu.mult)
            nc.scalar.sqrt(sth_[:], sth_[:])
            nc.vector.tensor_scalar(sg_[:], sphi_[:], 0.0, None, Alu.is_ge)
            nc.vector.tensor_scalar(sg_[:], sg_[:], 2.0, -1.0, Alu.mult, Alu.add)
            nc.vector.tensor_tensor(out=sth_[:], in0=sth_[:], in1=sg_[:], op=Alu.mult)
            res = dict(cx=cx_, cy=cy_, cth=cth_, sth=sth_, a=a_, b=b_)
            if not want_roots:
                return res
            Aa = nt("Aa"); Bb = nt("Bb")
            nc.vector.tensor_scalar(Aa[:], a_[:], float(EPS), None, Alu.add)
            nc.vector.tensor_scalar(Bb[:], b_[:], float(EPS), None, Alu.add)
            iA2 = nt("iA2"); iB2 = nt("iB2")
            nc.vector.tensor_tensor(out=gd[:], in0=Aa[:], in1=Aa[:], op=Alu.mult)
            nc.vector.reciprocal(iA2[:], gd[:])
            nc.vector.tensor_tensor(out=gd[:], in0=Bb[:], in1=Bb[:], op=Alu.mult)
            nc.vector.reciprocal(iB2[:], gd[:])
            c2t = nt("c2t"); s2t = nt("s2t")
            nc.vector.tensor_tensor(out=c2t[:], in0=cth_[:], in1=cth_[:], op=Alu.mult)
            nc.vector.tensor_tensor(out=s2t[:], in0=sth_[:], in1=sth_[:], op=Alu.mult)
            Pq = nt("Pq")
            nc.vector.tensor_tensor(out=Pq[:], in0=c2t[:], in1=iA2[:], op=Alu.mult)
            nc.vector.tensor_tensor(out=tz[:], in0=s2t[:], in1=iB2[:], op=Alu.mult)
            nc.vector.tensor_tensor(out=Pq[:], in0=Pq[:], in1=tz[:], op=Alu.add)
            Rq = nt("Rq")
            nc.vector.tensor_tensor(out=Rq[:], in0=iA2[:], in1=iB2[:], op=Alu.subtract)
            nc.vector.tensor_tensor(out=Rq[:], in0=Rq[:], in1=cth_[:], op=Alu.mult)
            nc.vector.tensor_tensor(out=Rq[:], in0=Rq[:], in1=sth_[:], op=Alu.mult)
            K3 = nt("K3")
            nc.vector.tensor_tensor(out=K3[:], in0=iA2[:], in1=iB2[:], op=Alu.mult)
            iP = nt("iP")
            nc.vector.reciprocal(iP[:], Pq[:])
            K1 = nt("K1")
            nc.vector.tensor_tensor(out=K1[:], in0=Rq[:], in1=iP[:], op=Alu.mult)
            M0 = nt("M0")
            nc.vector.tensor_tensor(out=M0[:], in0=K1[:], in1=cy_[:], op=Alu.mult)
            nc.vector.tensor_tensor(out=M0[:], in0=M0[:], in1=cx_[:], op=Alu.add)
            H2 = nt("H2")
            nc.vector.tensor_scalar(H2[:], K3[:], -1.0, None, Alu.mult)
            H1 = nt("H1")
            nc.vector.tensor_tensor(out=H1[:], in0=K3[:], in1=cy_[:], op=Alu.mult)
            nc.vector.tensor_scalar(H1[:], H1[:], 2.0, None, Alu.mult)
            H0 = nt("H0")
            nc.vector.tensor_tensor(out=tz[:], in0=H1[:], in1=cy_[:], op=Alu.mult)
            nc.vector.tensor_scalar(tz[:], tz[:], 0.5, None, Alu.mult)
            nc.vector.tensor_tensor(out=H0[:], in0=Pq[:], in1=tz[:], op=Alu.subtract)
            res.update(M0=M0, K1=K1, H0=H0, H1=H1, H2=H2, iP=iP)
            return res

        pt = fit_params(Wm, "pt", NL, True)

        # ======== back-broadcast coef rows to [128, SL2] (l-major, 65-slot) ========
        BS = med.tile([NS, 30], F32)
        for qi, q in enumerate([pt["M0"], pt["K1"], pt["H0"], pt["H1"], pt["H2"], pt["iP"]]):
            nc.vector.tensor_copy(BS[:, qi * 5:qi * 5 + 5], q[:])
        psb2 = ps1.tile([30, NS], F32, tag="psmisc")
        nc.tensor.transpose(psb2[:], BS[:, :], eye128[0:NS, 0:NS])
        BT = med.tile([30, NS], F32)
        nc.vector.tensor_copy(BT[:], psb2[:])
        # benign dummy-slot values: arg = -1 -> invalid -> cnt 0 -> idx -1
        # BS col order: [M0, K1, H0, H1, H2, iP] -> BT rows qi*5+l
        # Hrow3 rows: H2, H1, H0 (dummies 0, 0, -1); Mrow2 rows: M0, K1 (dummies 0)
        # dummy slots stay 0 -> they produce a benign 1-px interval whose
        # contributions land only in dummy columns, sliced out downstream
        Hrow3 = med.tile([3, SL2], F32)
        nc.gpsimd.memset(Hrow3[:, :], 0.0)
        for ri, qi in ((0, 4), (1, 3), (2, 2)):
            nc.sync.dma_start(
                Hrow3[ri:ri + 1, :].rearrange("p (l s) -> p l s", l=NL)[:, :, 0:NS],
                BT[qi * 5:qi * 5 + 5, :])
        Mrow2 = med.tile([2, SL2], F32)
        nc.gpsimd.memset(Mrow2[:, :], 0.0)
        for ri, qi in ((0, 0), (1, 1)):
            nc.sync.dma_start(
                Mrow2[ri:ri + 1, :].rearrange("p (l s) -> p l s", l=NL)[:, :, 0:NS],
                BT[qi * 5:qi * 5 + 5, :])
        rowip = sml.tile([1, SL2], F32, name="rowiP")
        nc.gpsimd.memset(rowip[:], 0.0)
        nc.sync.dma_start(rowip[:].rearrange("p (l s) -> p l s", l=NL)[:, :, 0:NS],
                          BT[25:30, :])
        bciP = med.tile([128, SL2], F32)
        nc.gpsimd.partition_broadcast(bciP[:], rowip[:], channels=128)

        # ================= roots [128, SL2] =================
        psarg = ps1.tile([128, SL2], F32, tag="psarg")
        nc.tensor.matmul(psarg[:], Y3[:], Hrow3[:], start=True, stop=True)
        valid = med.tile([128, SL2], F32)
        nc.vector.tensor_scalar(valid[:], psarg[:], 0.0, None, Alu.is_ge)
        arg = med.tile([128, SL2], F32)
        nc.vector.tensor_scalar(arg[:], psarg[:], 0.0, None, Alu.max)
        rt = med.tile([128, SL2], F32)
        nc.scalar.sqrt(rt[:], arg[:])
        rrec = med.tile([128, SL2], F32, tag="tq2")
        nc.vector.tensor_scalar(rrec[:], rt[:], 1e-30, None, Alu.max)
        nc.vector.reciprocal(rrec[:], rrec[:])
        nc.vector.tensor_tensor(out=rrec[:], in0=arg[:], in1=rrec[:], op=Alu.mult)
        nc.vector.tensor_tensor(out=rt[:], in0=rt[:], in1=rrec[:], op=Alu.add)
        nc.vector.tensor_scalar(rt[:], rt[:], 0.5, None, Alu.mult)
        half = rt
        nc.vector.tensor_tensor(out=half[:], in0=rt[:], in1=bciP[:], op=Alu.mult)
        psmrow = ps1.tile([128, SL2], F32, tag="psarg")
        nc.tensor.matmul(psmrow[:], Y2[:], Mrow2[:], start=True, stop=True)
        xlo = med.tile([128, SL2], F32)
        nc.vector.tensor_tensor(out=xlo[:], in0=psmrow[:], in1=half[:], op=Alu.subtract)
        nc.vector.tensor_scalar(xlo[:], xlo[:], 0.0, 127.0, Alu.max, Alu.min)
        xhi = med.tile([128, SL2], F32)
        nc.vector.tensor_tensor(out=xhi[:], in0=psmrow[:], in1=half[:], op=Alu.add)
        nc.vector.tensor_scalar(xhi[:], xhi[:], 0.0, 127.0, Alu.max, Alu.min)
        # nhi = floor(xhi), nlo = ceil(xlo) via int truncation
        nint = med.tile([128, SL2], I32, tag="nint")
        nc.vector.tensor_copy(nint[:], xhi[:])
        nhi = med.tile([128, SL2], F32)
        nc.vector.tensor_copy(nhi[:], nint[:])
        fhi = med.tile([128, SL2], F32, tag="tq3")
        nc.vector.tensor_tensor(out=fhi[:], in0=nhi[:], in1=xhi[:], op=Alu.is_gt)
        nc.vector.tensor_tensor(out=nhi[:], in0=nhi[:], in1=fhi[:], op=Alu.subtract)
        nc.vector.tensor_copy(nint[:], xlo[:])
        nlo = med.tile([128, SL2], F32)
        nc.vector.tensor_copy(nlo[:], nint[:])
        frac = med.tile([128, SL2], F32, tag="tq3")
        nc.vector.tensor_tensor(out=frac[:], in0=xlo[:], in1=nlo[:], op=Alu.is_gt)
        nc.vector.tensor_tensor(out=nlo[:], in0=nlo[:], in1=frac[:], op=Alu.add)
        cnt = med.tile([128, SL2], F32)
        nc.vector.tensor_tensor(out=cnt[:], in0=nhi[:], in1=nlo[:], op=Alu.subtract)
        nc.vector.tensor_scalar(cnt[:], cnt[:], 1.0, 0.0, Alu.add, Alu.max)
        nc.vector.tensor_tensor(out=cnt[:], in0=cnt[:], in1=valid[:], op=Alu.mult)
        okm = med.tile([128, SL2], F32, tag="tq")
        nc.vector.tensor_scalar(okm[:], cnt[:], 0.5, None, Alu.is_ge)

        # markers: idx = ok ? (n + 130*sl) : -1, interleaved (lo, hi) pairs
        M16 = med.tile([128, SL2 * 2], I16)
        m16v = M16[:].rearrange("p (q two) -> p q two", two=2)
        tmod = med.tile([128, SL2], F32, tag="tq2")
        nc.vector.tensor_tensor(out=tmod[:], in0=nlo[:], in1=LB1[:], op=Alu.add)
        nc.vector.tensor_tensor(out=tmod[:], in0=tmod[:], in1=okm[:], op=Alu.mult)
        nc.vector.tensor_scalar(tmod[:], tmod[:], 1.0, None, Alu.subtract)
        nc.vector.tensor_copy(m16v[:, :, 0], tmod[:])
        nc.vector.tensor_scalar(nhi[:], nhi[:], 1.0, None, Alu.add)
        nc.vector.tensor_tensor(out=tmod[:], in0=nhi[:], in1=LB1[:], op=Alu.add)
        nc.vector.tensor_tensor(out=tmod[:], in0=tmod[:], in1=okm[:], op=Alu.mult)
        nc.vector.tensor_scalar(tmod[:], tmod[:], 1.0, None, Alu.subtract)
        nc.vector.tensor_copy(m16v[:, :, 1], tmod[:])

        # ================= marker scatter + I1 =================
        # scatter +-1 markers, multiply by P2 prefix in place, fold segment
        # halves (130 -> 65) in bf16, then reduce the folded half-segments.
        red = med.tile([128, SL2], F32, name="red")
        fold = big.tile([128, NSLOT * 65], F32, name="fold", tag="bigA")
        for l in range(NL):
            D = dpool.tile([128, DW], BF16, name=f"D{l}", tag="dtile")
            for w in range(NW):
                base = (l * NW + w) * (SPW * 2)
                nc.gpsimd.local_scatter(
                    D[:, w * WELEM:(w + 1) * WELEM],
                    data26[:],
                    M16[:, base:base + SPW * 2],
                    channels=128, num_elems=WELEM, num_idxs=SPW * 2)
            nc.vector.tensor_tensor(out=D[:], in0=D[:], in1=P2[:], op=Alu.mult)
            Dv = D[:].rearrange("p (s c) -> p s c", s=NSLOT)
            fv65 = fold[:].rearrange("p (s c) -> p s c", s=NSLOT)
            nc.vector.tensor_tensor(out=fv65, in0=Dv[:, :, 0:65],
                                    in1=Dv[:, :, 65:130], op=Alu.add)
            nc.vector.tensor_reduce(
                red[:, l * NSLOT:(l + 1) * NSLOT], fv65, AX.X, Alu.add)

        psI1 = ps1.tile([1, SL2], F32, tag="psI1")
        nc.tensor.matmul(psI1[:], onescol[:], red[:], start=True, stop=True)
        psI0 = ps1.tile([1, SL2], F32, tag="psb")
        nc.tensor.matmul(psI0[:], onescol[:], cnt[:], start=True, stop=True)

        # ================= metric + argmax (on [1, SL] (l,64) layout) ==============
        SL = NL * NS
        I1r = sml.tile([1, SL], F32)
        nc.vector.tensor_copy(I1r[:].rearrange("p (l s) -> p l s", l=NL),
                              psI1[:].rearrange("p (l s) -> p l s", l=NL)[:, :, 0:NS])
        I0r = sml.tile([1, SL], F32)
        nc.vector.tensor_copy(I0r[:].rearrange("p (l s) -> p l s", l=NL),
                              psI0[:].rearrange("p (l s) -> p l s", l=NL)[:, :, 0:NS])
        iin = sml.tile([1, SL], F32)
        nc.vector.tensor_scalar(iin[:], I0r[:], 0.5, None, Alu.mult)
        nc.vector.tensor_tensor(out=iin[:], in0=iin[:], in1=I1r[:], op=Alu.add)
        met = sml.tile([1, SL], F32)
        nc.vector.tensor_tensor(out=met[:].rearrange("p (l s) -> p s l", l=NL),
                                in0=iin[:].rearrange("p (l s) -> p s l", l=NL),
                                in1=itr[:].to_broadcast((1, NS, NL)), op=Alu.mult)
        nc.vector.tensor_scalar(I0r[:], I0r[:], float(1.0 / NPIX), None, Alu.mult)
        nc.vector.tensor_tensor(out=met[:], in0=met[:], in1=I0r[:], op=Alu.subtract)
        mmax = sml.tile([1, NS], F32)
        nc.vector.tensor_reduce(mmax[:], met[:].rearrange("p (l s) -> p s l", l=NL), AX.X, Alu.max)
        lidx_i = sml.tile([1, SL], I32)
        nc.gpsimd.iota(lidx_i[:].rearrange("p (l s) -> p l s", l=NL),
                       pattern=[[1, NL], [0, NS]], base=0, channel_multiplier=0)
        cand = sml.tile([1, SL], F32)
        nc.vector.tensor_copy(cand[:], lidx_i[:])
        eqmax = sml.tile([1, SL], F32)
        nc.vector.tensor_tensor(out=eqmax[:].rearrange("p (l s) -> p s l", l=NL),
                                in0=met[:].rearrange("p (l s) -> p s l", l=NL),
                                in1=mmax[:].to_broadcast((1, NS, NL)), op=Alu.is_lt)
        nc.vector.tensor_scalar(eqmax[:], eqmax[:], 99.0, None, Alu.mult)
        nc.vector.tensor_tensor(out=cand[:], in0=cand[:], in1=eqmax[:], op=Alu.add)
        bestr = sml.tile([1, NS], F32)
        nc.vector.tensor_reduce(bestr[:], cand[:].rearrange("p (l s) -> p s l", l=NL), AX.X, Alu.min)

        # ================= output side =================
        lvlfr = sml.tile([1, NS], F32)
        nc.vector.tensor_scalar(lvlfr[:], bestr[:], 0.1, 0.3, Alu.mult, Alu.add)
        lvlfb = med.tile([128, NS], F32)
        nc.gpsimd.partition_broadcast(lvlfb[:], lvlfr[:], channels=128)
        taub = med.tile([128, NS], F32)
        nc.vector.tensor_tensor(out=taub[:], in0=lvlfb[:], in1=rngbO[:], op=Alu.mult)
        nc.vector.tensor_tensor(out=taub[:], in0=taub[:], in1=mnbO[:], op=Alu.add)

        # d = vo - taub (f32), then fo = Relu(d), go = Sign(d) on the Act engine
        dfield = big.tile([128, F], F32, name="dfield", tag="bigA")
        nc.vector.tensor_tensor(out=dfield[:].rearrange("p (s x) -> p s x", s=NS),
                                in0=vo[:].rearrange("p (s x) -> p s x", s=NS),
                                in1=taub[:].to_broadcast((128, NS, W)), op=Alu.subtract)
        fo = dpool.tile([128, DW], BF16, name="fo", tag="dtile")
        nc.scalar.activation(fo[:, 0:F], dfield[:], Act.Relu, bias=0.0, scale=1.0)
        go = dpool.tile([128, DW], BF16, name="go", tag="dtile")
        nc.vector.tensor_scalar(go[:, 0:F], dfield[:], 0.0, None, Alu.is_ge)

        SB = med.tile([NS, 12], F32)
        moments(fo, 100, SB, 0)
        moments(go, 101, SB, 6)

        def row2col(rowap, nm):
            pr = ps1.tile([NS, 1], F32, name=f"pr{nm}", tag="psmisc")
            nc.tensor.transpose(pr[:], rowap, eye128[0:1, 0:1])
            c = med.tile([NS, 1], F32, name=f"col{nm}")
            nc.vector.tensor_copy(c[:], pr[:])
            return c

        bestc = row2col(bestr[:], "best")
        mnoc = mncO
        rngoc = rngcO

        def col(nm):
            return med.tile([NS, 1], F32, name=nm)

        lvfc = col("lvfc")
        nc.vector.tensor_scalar(lvfc[:], bestc[:], 0.1, 0.3, Alu.mult, Alu.add)
        tauc = col("tauc")
        nc.vector.tensor_tensor(out=tauc[:], in0=lvfc[:], in1=rngoc[:], op=Alu.mult)
        nc.vector.tensor_tensor(out=tauc[:], in0=tauc[:], in1=mnoc[:], op=Alu.add)
        tmn = col("tmn")
        nc.vector.tensor_tensor(out=tmn[:], in0=tauc[:], in1=mnoc[:], op=Alu.subtract)
        WallO = med.tile([NS, 6], F32, name="WallO")
        nc.vector.tensor_scalar(WallO[:], SB[:, 6:12], tmn[:], None, Alu.mult)
        nc.vector.tensor_tensor(out=WallO[:], in0=WallO[:], in1=SB[:, 0:6], op=Alu.add)
        WmO = {j: WallO[:, j:j + 1] for j in range(6)}

        po = fit_params(WmO, "po", 1, False, refine=False)

        # select target params at best level
        eqm = med.tile([NS, NL], F32, name="eqm")
        l5f = med.tile([NS, NL], F32, name="l5f")
        nc.vector.tensor_copy(l5f[:], lr_i[:])
        nc.vector.tensor_scalar(eqm[:], l5f[:], bestc[:], None, Alu.is_equal)

        def select(src, nm):
            o = med.tile([NS, 1], F32, name="sel" + nm)
            tmp = med.tile([NS, NL], F32, name="selt" + nm, tag="seltmp")
            nc.vector.tensor_tensor(out=tmp[:], in0=src[:], in1=eqm[:], op=Alu.mult)
            nc.vector.tensor_reduce(o[:], tmp[:], AX.X, Alu.add)
            return o

        cxT = select(pt["cx"], "cx"); cyT = select(pt["cy"], "cy")
        cthT = select(pt["cth"], "ct"); sthT = select(pt["sth"], "st")
        aT = select(pt["a"], "a"); bT = select(pt["b"], "b")

        # ================= sym loss =================
        sc = col("sc")
        nc.vector.tensor_tensor(out=sc[:], in0=po["a"][:], in1=po["b"][:], op=Alu.max)
        t1c = col("t1c")
        nc.vector.tensor_tensor(out=t1c[:], in0=aT[:], in1=bT[:], op=Alu.max)
        nc.vector.tensor_tensor(out=sc[:], in0=sc[:], in1=t1c[:], op=Alu.max)
        nc.vector.tensor_scalar(sc[:], sc[:], float(EPS), None, Alu.add)
        isc = col("isc")
        nc.vector.reciprocal(isc[:], sc[:])
        lossc = col("lossc")
        td = col("td")

        def sqdiff_acc(xo, xt, first=False):
            nc.vector.tensor_tensor(out=td[:], in0=xo, in1=xt, op=Alu.subtract)
            nc.vector.tensor_tensor(out=td[:], in0=td[:], in1=isc[:], op=Alu.mult)
            nc.vector.tensor_tensor(out=td[:], in0=td[:], in1=td[:], op=Alu.mult)
            if first:
                nc.vector.tensor_copy(lossc[:], td[:])
            else:
                nc.vector.tensor_tensor(out=lossc[:], in0=lossc[:], in1=td[:], op=Alu.add)

        sqdiff_acc(po["cx"][:], cxT[:], first=True)
        sqdiff_acc(po["cy"][:], cyT[:])
        sqdiff_acc(po["a"][:], aT[:])
        sqdiff_acc(po["b"][:], bT[:])
        nc.vector.tensor_scalar(lossc[:], lossc[:], 0.5, None, Alu.mult)
        csum = col("csum")
        nc.vector.tensor_tensor(out=csum[:], in0=po["cth"][:], in1=cthT[:], op=Alu.mult)
        nc.vector.tensor_tensor(out=td[:], in0=po["sth"][:], in1=sthT[:], op=Alu.mult)
        nc.vector.tensor_tensor(out=csum[:], in0=csum[:], in1=td[:], op=Alu.add)
        nc.vector.tensor_scalar(csum[:], csum[:], -1.0, 1.0, Alu.mult, Alu.add)
        nc.vector.tensor_tensor(out=lossc[:], in0=lossc[:], in1=csum[:], op=Alu.add)

        nc.sync.dma_start(loss_out[:, :], lossc[:])


def build(NS=64, num_devices=1):
    nc = bacc.Bacc("TRN2", target_bir_lowering=False, debug=False, num_devices=num_devices)
    with tile.TileContext(nc) as tc:
        emit(nc, tc, NS=NS)
    nc.compile()
    return nc


_CACHED = {}


def _get_nc():
    if "nc" not in _CACHED:
        _CACHED["nc"] = build(NS=64, num_devices=8)
    return _CACHED["nc"]


def _make_in_maps(output, target, n_cores=8):
    output = np.ascontiguousarray(output, dtype=np.float32)
    target = np.ascontiguousarray(target, dtype=np.float32)
    per = output.shape[0] // n_cores
    in_maps = []
    for c in range(n_cores):
        sl = slice(c * per, (c + 1) * per)
        in_maps.append({
            "t": np.ascontiguousarray(target[sl].transpose(1, 0, 2)),
            "o": np.ascontiguousarray(output[sl].transpose(1, 0, 2)),
        })
    return in_maps


def kernel(output, target):
    from concourse.bass_utils import run_bass_kernel_spmd

    nc = _get_nc()
    in_maps = _make_in_maps(output, target)
    res = run_bass_kernel_spmd(nc, in_maps, core_ids=list(range(8)))
    losses = np.concatenate([r["loss"].reshape(-1) for r in res.results])
    return np.float32(losses.mean(dtype=np.float64))


# revision 26
# speedup vs baseline: 1.1247x; 1.1247x over previous
"""Trainium2 Bass kernel v3 for nn_BestEllipseLoss_5720896438361.

v3 over v2: fields moved to ACT engine (Relu/Sign with constant bias from a
bf16 normalized image), moment matmuls regrouped to 16x(N=512, M=96),
interval-sum reduce split into a bf16 halving fold + smaller f32-out reduce,
P2 memset shrunk to segment boundary columns, pure-f32 output-side dfield.

kernel(output, target): full [512,128,128] f32 inputs -> scalar f32 loss.
Shards batch across 8 NeuronCores (64 samples each), one SPMD Bass kernel.
"""
import sys
if "/opt/trn_rl_repo" not in sys.path:
    sys.path.insert(0, "/opt/trn_rl_repo")

import numpy as np

import concourse.bass as bass
import concourse.bacc as bacc
import concourse.tile as tile
import concourse.mybir as mybir
import concourse.bass_isa as bass_isa
import bass_rust

F32 = mybir.dt.float32
BF16 = mybir.dt.bfloat16
I32 = mybir.dt.int32
I16 = mybir.dt.int16
Alu = mybir.AluOpType
Act = mybir.ActivationFunctionType
AX = mybir.AxisListType

EPS = np.float32(1e-8)
LEVELS = [np.float32(0.3), np.float32(0.4), np.float32(0.5), np.float32(0.6), np.float32(0.7)]
NL = 5
H = 128
W = 128
NPIX = float(H * W)

_x = np.arange(W, dtype=np.float64)
_y = np.arange(H, dtype=np.float64)
# basis order j: {1, y, y^2, x, x*y, x^2}
C_B = np.array([
    H * W, W * _y.sum(), W * (_y ** 2).sum(),
    H * _x.sum(), _x.sum() * _y.sum(), H * (_x ** 2).sum(),
], dtype=np.float64).astype(np.float32)

# marker-scatter geometry
SEG = 130            # per-sample segment width in D / P2
SPW = 13             # samples per scatter window (SPW*SEG <= 2046)
NW = 5               # windows
NSLOT = SPW * NW     # 65 slots (64 real + 1 dummy)
DW = NSLOT * SEG     # 8450
WELEM = SPW * SEG    # 1690 elements per scatter call

# moment matmul grouping: x-chunk 8, basis j=12 (hi/lo), M=96, N=512
XC = 8               # x positions per group
NG = W // XC         # 16 groups
MROW = XC * 12       # 96 stationary rows


def _scan3d(eng, out, data0, data1, initial, op0, op1):
    """tensor_tensor_scan with multi-free-dim APs (recurrence chains across
    slices; data0 mask handles segment resets)."""
    return eng.add_instruction(
        mybir.InstTensorScalarPtr(
            name=eng.bass.get_next_instruction_name(),
            is_tensor_tensor_scan=True,
            is_scalar_tensor_tensor=True,
            op0=op0,
            op1=op1,
            ins=[
                eng.lower_ap(data0),
                eng.lower_ap_or_imm(initial),
                eng.lower_ap(data1),
            ],
            outs=[eng.lower_ap(out)],
        )
    )


def _ovl(ap, nslot, seg, stride):
    """Overlapping windowed view [(p), nslot, seg] with window stride."""
    v = ap.copy()
    v.ap = bass_rust.VecI64Pair([list(ap.ap[0]), [stride, nslot], [1, seg]])
    return v


def emit(nc, tc, NS=64):
    F = NS * W
    SL2 = NL * NSLOT      # 325, (l-major, 65-slot) layout

    t_in = nc.dram_tensor("t", [H, NS, W], F32, kind="ExternalInput")
    o_in = nc.dram_tensor("o", [H, NS, W], F32, kind="ExternalInput")
    loss_out = nc.dram_tensor("loss", [NS, 1], F32, kind="ExternalOutput")

    with tc.tile_pool(name="big", bufs=1) as big, \
         tc.tile_pool(name="dpool", bufs=2) as dpool, \
         tc.tile_pool(name="fg", bufs=1) as fg, \
         tc.tile_pool(name="med", bufs=1) as med, \
         tc.tile_pool(name="sml", bufs=1) as sml, \
         tc.tile_pool(name="ps", bufs=2, space="PSUM") as ps, \
         tc.tile_pool(name="ps1", bufs=1, space="PSUM") as ps1:

        # ================= constants =================
        yi = sml.tile([128, 1], I32)
        nc.gpsimd.iota(yi[:], pattern=[[0, 1]], base=0, channel_multiplier=1)
        yv = sml.tile([128, 1], F32)
        nc.vector.tensor_copy(yv[:], yi[:])
        y2v = sml.tile([128, 1], F32)
        nc.vector.tensor_tensor(out=y2v[:], in0=yv[:], in1=yv[:], op=Alu.mult)

        ei = med.tile([128, 128], I32, tag="scrA")
        nc.gpsimd.iota(ei[:], pattern=[[1, 128]], base=0, channel_multiplier=-1)
        eif = med.tile([128, 128], F32, tag="scrB")
        nc.vector.tensor_copy(eif[:], ei[:])
        eye128 = med.tile([128, 128], F32)
        nc.vector.tensor_scalar(eye128[:], eif[:], 0.0, None, Alu.is_equal)

        onescol = sml.tile([128, 1], F32)
        nc.gpsimd.memset(onescol[:], 1.0)
        onescolb = sml.tile([128, 1], BF16)
        nc.gpsimd.memset(onescolb[:], 1.0)

        lvl_bias = []
        for l in range(NL):
            b = sml.tile([128, 1], F32, name=f"lvlb{l}")
            nc.gpsimd.memset(b[:], -float(LEVELS[l] - np.float32(0.5)))
            lvl_bias.append(b)

        # SEL_i [96, 6] fold matrices: pick q = 12*i + j (hi) and 12*i+6+j (lo)
        di = med.tile([MROW, 6], I32, tag="scrC")
        nc.gpsimd.iota(di[:], pattern=[[-1, 6]], base=0, channel_multiplier=1)
        df = med.tile([MROW, 6], F32, tag="scrD")
        nc.vector.tensor_copy(df[:], di[:])
        SELS = []
        for i in range(XC):
            s1 = med.tile([MROW, 6], F32, name=f"sel{i}")
            nc.vector.tensor_scalar(s1[:], df[:], float(12 * i), None, Alu.is_equal)
            s2 = med.tile([MROW, 6], F32, name=f"sel2_{i}", tag="scrE")
            nc.vector.tensor_scalar(s2[:], df[:], float(12 * i + 6), None, Alu.is_equal)
            nc.vector.tensor_tensor(out=s1[:], in0=s1[:], in1=s2[:], op=Alu.add)
            SELS.append(s1)

        # ---- moment lhsT table [128, NG*MROW] bf16 (hi/lo split basis) ----
        mast = med.tile([128, 768], F32, tag="scrA")
        nc.gpsimd.memset(mast[:], 1.0)
        xri = sml.tile([1, W], I32)
        nc.gpsimd.iota(xri[:], pattern=[[1, W]], base=0, channel_multiplier=0)
        xrf = sml.tile([1, W], F32)
        nc.vector.tensor_copy(xrf[:], xri[:])
        x2rf = sml.tile([1, W], F32)
        nc.vector.tensor_tensor(out=x2rf[:], in0=xrf[:], in1=xrf[:], op=Alu.mult)
        xfull = med.tile([128, W], F32, tag="scrB")
        nc.gpsimd.partition_broadcast(xfull[:], xrf[:], channels=128)
        x2full = med.tile([128, W], F32, tag="scrF")
        nc.gpsimd.partition_broadcast(x2full[:], x2rf[:], channels=128)
        mv = mast[:].rearrange("p (g t j) -> p g t j", g=NG, t=XC)
        xfv = xfull[:].rearrange("p (g t) -> p g t", g=NG)
        x2fv = x2full[:].rearrange("p (g t) -> p g t", g=NG)
        nc.vector.tensor_copy(mv[:, :, :, 3:4], xfv.to_broadcast((128, NG, XC, 1)))
        nc.vector.tensor_copy(mv[:, :, :, 4:5], xfv.to_broadcast((128, NG, XC, 1)))
        nc.vector.tensor_copy(mv[:, :, :, 5:6], x2fv.to_broadcast((128, NG, XC, 1)))
        mgt = mast[:].rearrange("p (gt j) -> p gt j", j=6)
        nc.vector.tensor_scalar(mgt[:, :, 1:2], mgt[:, :, 1:2], yv[:], None, Alu.mult)
        nc.vector.tensor_scalar(mgt[:, :, 4:5], mgt[:, :, 4:5], yv[:], None, Alu.mult)
        nc.vector.tensor_scalar(mgt[:, :, 2:3], mgt[:, :, 2:3], y2v[:], None, Alu.mult)
        hi24 = med.tile([128, 768], BF16, tag="scrG")
        nc.vector.tensor_copy(hi24[:], mast[:])
        lo24 = mast
        nc.vector.tensor_tensor(out=lo24[:], in0=mast[:], in1=hi24[:], op=Alu.subtract)
        table = med.tile([128, NG * MROW], BF16)
        tvv = table[:].rearrange("p (g t j) -> p g t j", g=NG, t=XC)
        nc.vector.tensor_copy(tvv[:, :, :, 0:6], hi24[:].rearrange("p (g t j) -> p g t j", g=NG, t=XC))
        nc.vector.tensor_copy(tvv[:, :, :, 6:12], lo24[:].rearrange("p (g t j) -> p g t j", g=NG, t=XC))

        # lvl rows [NS, NL]
        lr_i = sml.tile([NS, NL], I32)
        nc.gpsimd.iota(lr_i[:], pattern=[[1, NL]], base=0, channel_multiplier=0)

        # marker bases: LB1 = 130*sl + 1 over (l, w, sl)
        lb_i = med.tile([128, SL2], I32, tag="nint")
        nc.gpsimd.iota(lb_i[:].rearrange("p (l w s) -> p l w s", l=NL, w=NW),
                       pattern=[[0, NL], [0, NW], [SEG, SPW]], base=1,
                       channel_multiplier=0)
        LB1 = med.tile([128, SL2], F32)
        nc.vector.tensor_copy(LB1[:], lb_i[:])
        # rank-2 lhsT for arg/mrow matmuls: Y3 = [y^2; y; 1], Y2 = [1; -y]
        y3c = med.tile([128, 3], F32, tag="scrD2")
        nc.vector.tensor_copy(y3c[:, 0:1], y2v[:])
        nc.vector.tensor_copy(y3c[:, 1:2], yv[:])
        nc.vector.tensor_scalar(y3c[:, 2:3], yv[:], 0.0, 1.0, Alu.mult, Alu.add)
        psy = ps1.tile([3, 128], F32, tag="psmisc")
        nc.tensor.transpose(psy[:], y3c[:], eye128[:, :])
        Y3 = med.tile([3, 128], F32)
        nc.vector.tensor_copy(Y3[:], psy[:])
        y2c = med.tile([128, 2], F32, tag="scrD3")
        nc.vector.tensor_scalar(y2c[:, 0:1], yv[:], 0.0, 1.0, Alu.mult, Alu.add)
        nc.vector.tensor_scalar(y2c[:, 1:2], yv[:], -1.0, None, Alu.mult)
        psy2 = ps1.tile([2, 128], F32, tag="psmisc")
        nc.tensor.transpose(psy2[:], y2c[:], eye128[:, :])
        Y2 = med.tile([2, 128], F32)
        nc.vector.tensor_copy(Y2[:], psy2[:])

        # scatter data row: (-1, +1) x SPW in bf16
        d26i = sml.tile([128, SPW * 2], I32)
        nc.gpsimd.iota(d26i[:], pattern=[[0, SPW], [1, 2]], base=0, channel_multiplier=0)
        d26f = sml.tile([128, SPW * 2], F32)
        nc.vector.tensor_copy(d26f[:], d26i[:])
        data26 = sml.tile([128, SPW * 2], BF16)
        nc.vector.tensor_scalar(data26[:], d26f[:], 2.0, -1.0, Alu.mult, Alu.add)

        # ================= loads =================
        hp = tc.high_priority(offset=100000)
        hp.__enter__()
        # 4-queue split load: each engine's DGE processes its descriptor batch
        # concurrently, ~4x the single-queue descriptor rate
        vt = big.tile([128, F], F32, tag="bigA")
        tv_flat = t_in[:, :, :].rearrange("y s x -> y (s x)")
        tengs = (nc.sync, nc.scalar, nc.gpsimd, nc.sync)
        for i in range(4):
            tengs[i].dma_start(vt[:, i * (F // 4):(i + 1) * (F // 4)],
                               tv_flat[:, i * (F // 4):(i + 1) * (F // 4)])
        vo = big.tile([128, F], F32, tag="bigO", name="vo")
        ov_flat = o_in[:, :, :].rearrange("y s x -> y (s x)")
        for i in range(2):
            eng = (nc.scalar, nc.gpsimd)[i % 2]
            eng.dma_start(vo[:, i * (F // 2):(i + 1) * (F // 2)],
                          ov_flat[:, i * (F // 2):(i + 1) * (F // 2)])

        # ================= min/max + normalize (target) =================
        # per-partition max / negated min land side by side in mmT; one PE
        # transpose + free-dim reduce replaces gpsimd partition_all_reduce
        mmT = med.tile([128, 2 * NS], F32, name="mmT", tag="mxp")
        NH2 = NS // 2
        NQ = NS // 4
        for q in range(4):
            hv = vt[:, q * NQ * W:(q + 1) * NQ * W].rearrange("p (s x) -> p s x", s=NQ)
            nc.vector.tensor_reduce(mmT[:, q * NQ:(q + 1) * NQ], hv, AX.X, Alu.max)
            nc.vector.tensor_reduce(mmT[:, NS + q * NQ:NS + (q + 1) * NQ], hv, AX.X, Alu.min)
        nc.vector.tensor_scalar(mmT[:, NS:2 * NS], mmT[:, NS:2 * NS], -1.0, None, Alu.mult)
        # separate transposes rebase both halves to partition 0
        psmxT = ps1.tile([NS, 128], F32, tag="psarg")
        nc.tensor.transpose(psmxT[:], mmT[:, 0:NS], eye128[:, :])
        mxcT = med.tile([NS, 1], F32, name="mxcT")
        nc.vector.tensor_reduce(mxcT[:], psmxT[:], AX.X, Alu.max)
        psmnT = ps1.tile([NS, 128], F32, tag="psarg")
        nc.tensor.transpose(psmnT[:], mmT[:, NS:2 * NS], eye128[:, :])
        negmnT = med.tile([NS, 1], F32, name="negmnT")
        nc.vector.tensor_reduce(negmnT[:], psmnT[:], AX.X, Alu.max)
        rngcT = med.tile([NS, 1], F32, name="rngcT")
        nc.vector.tensor_tensor(out=rngcT[:], in0=mxcT[:], in1=negmnT[:], op=Alu.add)
        nc.vector.tensor_scalar(rngcT[:], rngcT[:], float(EPS), None, Alu.add)
        rbcT = med.tile([NS, 1], F32, name="rbcT")
        nc.vector.reciprocal(rbcT[:], rngcT[:])
        shcT = med.tile([NS, 1], F32, name="shcT")
        nc.vector.tensor_scalar(shcT[:], rngcT[:], 0.5, None, Alu.mult)
        nc.vector.tensor_tensor(out=shcT[:], in0=shcT[:], in1=negmnT[:], op=Alu.subtract)
        # rowify (shift, rb) -> [128, NS] broadcast rows
        def rowify(colap, bcast_out, nm):
            pr = ps1.tile([1, NS], F32, tag="psmisc", name=f"psr{nm}")
            nc.tensor.transpose(pr[:], colap, eye128[0:NS, 0:NS])
            r = med.tile([1, NS], F32, name=f"row{nm}")
            nc.vector.tensor_copy(r[:], pr[:])
            nc.gpsimd.partition_broadcast(bcast_out, r[:], channels=128)
            return r

        shiftT = med.tile([128, NS], F32)
        rowify(shcT[:], shiftT[:], "shT")
        rbT = med.tile([128, NS], F32)
        rowify(rbcT[:], rbT[:], "rbT")

        for hi2 in range(2):
            upv = vt[:, hi2 * NH2 * W:(hi2 + 1) * NH2 * W].rearrange("p (s x) -> p s x", s=NH2)
            sh = shiftT[:, hi2 * NH2:(hi2 + 1) * NH2]
            rb = rbT[:, hi2 * NH2:(hi2 + 1) * NH2]
            nc.vector.tensor_tensor(out=upv, in0=upv,
                                    in1=sh.to_broadcast((128, NH2, W)), op=Alu.subtract)
            nc.vector.tensor_tensor(out=upv, in0=upv,
                                    in1=rb.to_broadcast((128, NH2, W)), op=Alu.mult)
        # ================= fields + moments (target) =================
        SA = med.tile([NS, 72], F32)

        def moments_mm(field_tile, fl):
            psm = ps.tile([MROW, NS * XC], F32, name=f"psm{fl}", tag="psmom")
            fv = field_tile[:, 0:F].rearrange("p (s x) -> p s x", s=NS)
            for g in range(NG):
                nc.tensor.matmul(psm[:], table[:, g * MROW:(g + 1) * MROW],
                                 fv[:, :, XC * g:XC * g + XC],
                                 start=(g == 0), stop=(g == NG - 1))
            return psm

        def moments_fold(psm, fl, out_tile, col0):
            S96 = med.tile([MROW, NS * XC], F32, name=f"s96_{fl}", tag=f"s96{fl % 2}")
            nc.vector.tensor_copy(S96[:], psm[:])
            pT = ps.tile([NS, 6], F32, name=f"pT{fl}", tag="ps6")
            sv = S96[:].rearrange("q (s t) -> q s t", s=NS)
            for i in range(XC):
                nc.tensor.matmul(pT[:], sv[:, :, i], SELS[i][:],
                                 start=(i == 0), stop=(i == XC - 1))
            nc.vector.tensor_copy(out_tile[:, col0:col0 + 6], pT[:])

        def moments(field_tile, fl, out_tile, col0):
            moments_fold(moments_mm(field_tile, fl), fl, out_tile, col0)

        # deferred folds keep the PE matmul stream continuous (pstate ramp)
        for l in range(NL):
            lvlp = float(LEVELS[l] - np.float32(0.5))
            f_l = fg.tile([128, F], BF16, name=f"f{l}", tag="ffield")
            nc.vector.tensor_scalar(f_l[:], vt[:, 0:F], lvlp, lvlp, Alu.max, Alu.subtract)
            g_l = fg.tile([128, F], BF16, name=f"g{l}", tag="gfield")
            nc.scalar.activation(g_l[:], vt[:, 0:F], Act.Sign, bias=lvl_bias[l][:], scale=1.0)
            psf = moments_mm(f_l, l)
            psg = moments_mm(g_l, 5 + l)
            moments_fold(psf, l, SA, l * 6)
            moments_fold(psg, 5 + l, SA, 30 + l * 6)

        # ============ segmented prefix scan -> P2 (bf16, windowed) ============
        # state = (rowmask * state) + u' resets at each sample's x=0; output
        # lands at window slot x+1, so slot k = sum_{x<k} u' per sample.
        mi_ = sml.tile([128, W], I32, name="mi_")
        nc.gpsimd.iota(mi_[:], pattern=[[1, W]], base=0, channel_multiplier=0)
        mf_ = sml.tile([128, W], F32, name="mf_")
        nc.vector.tensor_copy(mf_[:], mi_[:])
        nc.vector.tensor_scalar(mf_[:], mf_[:], 0.5, None, Alu.is_ge)
        P2 = big.tile([128, DW], BF16, name="P2", tag="bigP")
        p2segs = P2[:].rearrange("p (s c) -> p s c", s=NSLOT)
        nc.gpsimd.memset(p2segs[:, :, 0:1], 0.0)
        nc.gpsimd.memset(p2segs[:, :, 129:130], 0.0)
        _scan3d(nc.vector,
                _ovl(P2[:, 1:], NS, W, SEG),
                _ovl(mf_[:, :], NS, W, 0),
                vt[:, 0:F].rearrange("p (s x) -> p s x", s=NS),
                0.0, Alu.mult, Alu.add)

        # itot from P2 column 128: itot(s) = sum_y P'[y, s, 128]
        p2col = P2[:].rearrange("p (s c) -> p s c", s=NSLOT)[:, :, 128]
        psb = ps1.tile([1, NSLOT], F32, tag="psb")
        nc.tensor.matmul(psb[:], onescolb[:], p2col, start=True, stop=True)
        brow = sml.tile([1, NSLOT], F32)
        nc.vector.tensor_copy(brow[:], psb[:])
        itot = sml.tile([1, NS], F32)
        nc.vector.tensor_scalar(itot[:], brow[:, 0:NS], 0.5 * NPIX, float(EPS), Alu.add, Alu.add)
        itr = sml.tile([1, NS], F32)
        nc.vector.reciprocal(itr[:], itot[:])

        hp.__exit__(None, None, None)

        # minmaxO per-half (slots into field-phase DVE gaps)
        # quarter-chunked so the scheduler can slot these into small DVE gaps
        mmO = med.tile([128, 2 * NS], F32, name="mmO", tag="mxp")
        for q in range(4):
            hv = vo[:, q * NQ * W:(q + 1) * NQ * W].rearrange("p (s x) -> p s x", s=NQ)
            nc.vector.tensor_reduce(mmO[:, q * NQ:(q + 1) * NQ], hv, AX.X, Alu.max)
            nc.vector.tensor_reduce(mmO[:, NS + q * NQ:NS + (q + 1) * NQ], hv, AX.X, Alu.min)
        nc.vector.tensor_scalar(mmO[:, NS:2 * NS], mmO[:, NS:2 * NS], -1.0, None, Alu.mult)
        psmxO = ps1.tile([NS, 128], F32, tag="psarg")
        nc.tensor.transpose(psmxO[:], mmO[:, 0:NS], eye128[:, :])
        mxcO = med.tile([NS, 1], F32, name="mxcO")
        nc.vector.tensor_reduce(mxcO[:], psmxO[:], AX.X, Alu.max)
        psmnO = ps1.tile([NS, 128], F32, tag="psarg")
        nc.tensor.transpose(psmnO[:], mmO[:, NS:2 * NS], eye128[:, :])
        negmnO = med.tile([NS, 1], F32, name="negmnO")
        nc.vector.tensor_reduce(negmnO[:], psmnO[:], AX.X, Alu.max)
        mncO = med.tile([NS, 1], F32, name="mncO")
        nc.vector.tensor_scalar(mncO[:], negmnO[:], -1.0, None, Alu.mult)
        rngcO = med.tile([NS, 1], F32, name="rngcO")
        nc.vector.tensor_tensor(out=rngcO[:], in0=mxcO[:], in1=negmnO[:], op=Alu.add)
        mnbO = med.tile([128, NS], F32, name="mnbO")
        rowify(mncO[:], mnbO[:], "mnO")
        rngbO = med.tile([128, NS], F32, name="rngbO")
        rowify(rngcO[:], rngbO[:], "rngO")

        # ================= target params (sample layout [NS, NL]) =================
        def FA(j):
            return SA[:, j:j + 30:6]

        def GA(j):
            return SA[:, 30 + j:30 + j + 30:6]

        cbt = med.tile([NS, 30], F32, name="cbt")
        hlvl = med.tile([NS, 30], F32, name="hlvl")
        for j in range(6):
            nc.gpsimd.memset(cbt[:, j:30:6], float(C_B[j]))
        for l in range(NL):
            nc.gpsimd.memset(hlvl[:, l * 6:(l + 1) * 6], 0.5 * float(LEVELS[l]))
        Wall = med.tile([NS, 30], F32, name="Wall")
        nc.vector.tensor_tensor(out=Wall[:], in0=SA[:, 30:60], in1=cbt[:], op=Alu.add)
        nc.vector.tensor_tensor(out=Wall[:], in0=Wall[:], in1=hlvl[:], op=Alu.mult)
        nc.vector.tensor_tensor(out=Wall[:], in0=Wall[:], in1=SA[:, 0:30], op=Alu.add)
        Wm = {j: Wall[:, j:30:6] for j in range(6)}

        def fit_params(Wd, tagp, n_l, want_roots, refine=True):
            def nt(nm):
                return med.tile([NS, n_l], F32, name=tagp + nm)
            m00_ = nt("m00")
            nc.vector.tensor_scalar(m00_[:], Wd[0], float(EPS), None, Alu.add)
            im_ = nt("im")
            nc.vector.reciprocal(im_[:], m00_[:])
            cx_ = nt("cx"); cy_ = nt("cy"); tz = nt("tz")
            nc.vector.tensor_tensor(out=cx_[:], in0=Wd[3], in1=im_[:], op=Alu.mult)
            nc.vector.tensor_tensor(out=cy_[:], in0=Wd[1], in1=im_[:], op=Alu.mult)
            mu20_ = nt("mu20"); mu02_ = nt("mu02"); mu11_ = nt("mu11")
            nc.vector.tensor_tensor(out=mu20_[:], in0=Wd[5], in1=im_[:], op=Alu.mult)
            nc.vector.tensor_tensor(out=tz[:], in0=cx_[:], in1=cx_[:], op=Alu.mult)
            nc.vector.tensor_tensor(out=mu20_[:], in0=mu20_[:], in1=tz[:], op=Alu.subtract)
            nc.vector.tensor_tensor(out=mu02_[:], in0=Wd[2], in1=im_[:], op=Alu.mult)
            nc.vector.tensor_tensor(out=tz[:], in0=cy_[:], in1=cy_[:], op=Alu.mult)
            nc.vector.tensor_tensor(out=mu02_[:], in0=mu02_[:], in1=tz[:], op=Alu.subtract)
            nc.vector.tensor_tensor(out=mu11_[:], in0=Wd[4], in1=im_[:], op=Alu.mult)
            nc.vector.tensor_tensor(out=tz[:], in0=cx_[:], in1=cy_[:], op=Alu.mult)
            nc.vector.tensor_tensor(out=mu11_[:], in0=mu11_[:], in1=tz[:], op=Alu.subtract)
            dmu_ = nt("dmu"); smu_ = nt("smu"); cc_ = nt("cc")
            nc.vector.tensor_tensor(out=dmu_[:], in0=mu20_[:], in1=mu02_[:], op=Alu.subtract)
            nc.vector.tensor_tensor(out=smu_[:], in0=mu20_[:], in1=mu02_[:], op=Alu.add)
            nc.vector.tensor_tensor(out=cc_[:], in0=dmu_[:], in1=dmu_[:], op=Alu.mult)
            nc.vector.tensor_tensor(out=tz[:], in0=mu11_[:], in1=mu11_[:], op=Alu.mult)
            nc.vector.tensor_scalar(tz[:], tz[:], 4.0, None, Alu.mult)
            nc.vector.tensor_tensor(out=cc_[:], in0=cc_[:], in1=tz[:], op=Alu.add)
            com_ = nt("com")
            nc.scalar.sqrt(com_[:], cc_[:])
            gd = nt("gd"); rc = nt("rc")
            if refine:
                nc.vector.tensor_scalar(gd[:], com_[:], 1e-30, None, Alu.max)
                nc.vector.reciprocal(rc[:], gd[:])
                nc.vector.tensor_tensor(out=rc[:], in0=cc_[:], in1=rc[:], op=Alu.mult)
                nc.vector.tensor_tensor(out=com_[:], in0=com_[:], in1=rc[:], op=Alu.add)
                nc.vector.tensor_scalar(com_[:], com_[:], 0.5, None, Alu.mult)
            a2_ = nt("a2"); b2_ = nt("b2")
            nc.vector.tensor_tensor(out=a2_[:], in0=smu_[:], in1=com_[:], op=Alu.add)
            nc.vector.tensor_scalar(a2_[:], a2_[:], 2.0, float(EPS), Alu.mult, Alu.max)
            nc.vector.tensor_tensor(out=b2_[:], in0=smu_[:], in1=com_[:], op=Alu.subtract)
            nc.vector.tensor_scalar(b2_[:], b2_[:], 2.0, float(EPS), Alu.mult, Alu.max)
            a_ = nt("a"); b_ = nt("b")
            nc.scalar.sqrt(a_[:], a2_[:])
            nc.scalar.sqrt(b_[:], b2_[:])
            if refine:
                nc.vector.tensor_scalar(gd[:], a_[:], 1e-30, None, Alu.max)
                nc.vector.reciprocal(rc[:], gd[:])
                nc.vector.tensor_tensor(out=rc[:], in0=a2_[:], in1=rc[:], op=Alu.mult)
                nc.vector.tensor_tensor(out=a_[:], in0=a_[:], in1=rc[:], op=Alu.add)
                nc.vector.tensor_scalar(a_[:], a_[:], 0.5, None, Alu.mult)
                nc.vector.tensor_scalar(gd[:], b_[:], 1e-30, None, Alu.max)
                nc.vector.reciprocal(rc[:], gd[:])
                nc.vector.tensor_tensor(out=rc[:], in0=b2_[:], in1=rc[:], op=Alu.mult)
                nc.vector.tensor_tensor(out=b_[:], in0=b_[:], in1=rc[:], op=Alu.add)
                nc.vector.tensor_scalar(b_[:], b_[:], 0.5, None, Alu.mult)
            cg = nt("cg"); ic = nt("ic")
            nc.vector.tensor_scalar(cg[:], com_[:], 1e-30, None, Alu.max)
            nc.vector.reciprocal(ic[:], cg[:])
            cphi_ = nt("cphi"); sphi_ = nt("sphi")
            nc.vector.tensor_tensor(out=cphi_[:], in0=dmu_[:], in1=ic[:], op=Alu.mult)
            nc.vector.tensor_scalar(cphi_[:], cphi_[:], -1.0, 1.0, Alu.max, Alu.min)
            nc.vector.tensor_tensor(out=sphi_[:], in0=mu11_[:], in1=ic[:], op=Alu.mult)
            cth_ = nt("cth"); sth_ = nt("sth"); sg_ = nt("sg")
            nc.vector.tensor_scalar(cth_[:], cphi_[:], 1.0, 0.5, Alu.add, Alu.mult)
            nc.scalar.sqrt(cth_[:], cth_[:])
            nc.vector.tensor_scalar(sth_[:], cphi_[:], -1.0, None, Alu.mult)
            nc.vector.tensor_scalar(sth_[:], sth_[:], 1.0, 0.5, Alu.add, Alu.mult)
            nc.scalar.sqrt(sth_[:], sth_[:])
            nc.vector.tensor_scalar(sg_[:], sphi_[:], 0.0, None, Alu.is_ge)
            nc.vector.tensor_scalar(sg_[:], sg_[:], 2.0, -1.0, Alu.mult, Alu.add)
            nc.vector.tensor_tensor(out=sth_[:], in0=sth_[:], in1=sg_[:], op=Alu.mult)
            res = dict(cx=cx_, cy=cy_, cth=cth_, sth=sth_, a=a_, b=b_)
            if not want_roots:
                return res
            Aa = nt("Aa"); Bb = nt("Bb")
            nc.vector.tensor_scalar(Aa[:], a_[:], float(EPS), None, Alu.add)
            nc.vector.tensor_scalar(Bb[:], b_[:], float(EPS), None, Alu.add)
            iA2 = nt("iA2"); iB2 = nt("iB2")
            nc.vector.tensor_tensor(out=gd[:], in0=Aa[:], in1=Aa[:], op=Alu.mult)
            nc.vector.reciprocal(iA2[:], gd[:])
            nc.vector.tensor_tensor(out=gd[:], in0=Bb[:], in1=Bb[:], op=Alu.mult)
            nc.vector.reciprocal(iB2[:], gd[:])
            c2t = nt("c2t"); s2t = nt("s2t")
            nc.vector.tensor_tensor(out=c2t[:], in0=cth_[:], in1=cth_[:], op=Alu.mult)
            nc.vector.tensor_tensor(out=s2t[:], in0=sth_[:], in1=sth_[:], op=Alu.mult)
            Pq = nt("Pq")
            nc.vector.tensor_tensor(out=Pq[:], in0=c2t[:], in1=iA2[:], op=Alu.mult)
            nc.vector.tensor_tensor(out=tz[:], in0=s2t[:], in1=iB2[:], op=Alu.mult)
            nc.vector.tensor_tensor(out=Pq[:], in0=Pq[:], in1=tz[:], op=Alu.add)
            Rq = nt("Rq")
            nc.vector.tensor_tensor(out=Rq[:], in0=iA2[:], in1=iB2[:], op=Alu.subtract)
            nc.vector.tensor_tensor(out=Rq[:], in0=Rq[:], in1=cth_[:], op=Alu.mult)
            nc.vector.tensor_tensor(out=Rq[:], in0=Rq[:], in1=sth_[:], op=Alu.mult)
            K3 = nt("K3")
            nc.vector.tensor_tensor(out=K3[:], in0=iA2[:], in1=iB2[:], op=Alu.mult)
            iP = nt("iP")
            nc.vector.reciprocal(iP[:], Pq[:])
            K1 = nt("K1")
            nc.vector.tensor_tensor(out=K1[:], in0=Rq[:], in1=iP[:], op=Alu.mult)
            M0 = nt("M0")
            nc.vector.tensor_tensor(out=M0[:], in0=K1[:], in1=cy_[:], op=Alu.mult)
            nc.vector.tensor_tensor(out=M0[:], in0=M0[:], in1=cx_[:], op=Alu.add)
            H2 = nt("H2")
            nc.vector.tensor_scalar(H2[:], K3[:], -1.0, None, Alu.mult)
            H1 = nt("H1")
            nc.vector.tensor_tensor(out=H1[:], in0=K3[:], in1=cy_[:], op=Alu.mult)
            nc.vector.tensor_scalar(H1[:], H1[:], 2.0, None, Alu.mult)
            H0 = nt("H0")
            nc.vector.tensor_tensor(out=tz[:], in0=H1[:], in1=cy_[:], op=Alu.mult)
            nc.vector.tensor_scalar(tz[:], tz[:], 0.5, None, Alu.mult)
            nc.vector.tensor_tensor(out=H0[:], in0=Pq[:], in1=tz[:], op=Alu.subtract)
            res.update(M0=M0, K1=K1, H0=H0, H1=H1, H2=H2, iP=iP)
            return res

        pt = fit_params(Wm, "pt", NL, True)

        # ======== back-broadcast coef rows to [128, SL2] (l-major, 65-slot) ========
        BS = med.tile([NS, 30], F32)
        for qi, q in enumerate([pt["M0"], pt["K1"], pt["H0"], pt["H1"], pt["H2"], pt["iP"]]):
            nc.vector.tensor_copy(BS[:, qi * 5:qi * 5 + 5], q[:])
        psb2 = ps1.tile([30, NS], F32, tag="psmisc")
        nc.tensor.transpose(psb2[:], BS[:, :], eye128[0:NS, 0:NS])
        BT = med.tile([30, NS], F32)
        nc.vector.tensor_copy(BT[:], psb2[:])
        # benign dummy-slot values: arg = -1 -> invalid -> cnt 0 -> idx -1
        # BS col order: [M0, K1, H0, H1, H2, iP] -> BT rows qi*5+l
        # Hrow3 rows: H2, H1, H0 (dummies 0, 0, -1); Mrow2 rows: M0, K1 (dummies 0)
        # dummy slots stay 0 -> they produce a benign 1-px interval whose
        # contributions land only in dummy columns, sliced out downstream
        Hrow3 = med.tile([3, SL2], F32)
        nc.gpsimd.memset(Hrow3[:, :], 0.0)
        for ri, qi in ((0, 4), (1, 3), (2, 2)):
            nc.sync.dma_start(
                Hrow3[ri:ri + 1, :].rearrange("p (l s) -> p l s", l=NL)[:, :, 0:NS],
                BT[qi * 5:qi * 5 + 5, :])
        Mrow2 = med.tile([2, SL2], F32)
        nc.gpsimd.memset(Mrow2[:, :], 0.0)
        for ri, qi in ((0, 0), (1, 1)):
            nc.sync.dma_start(
                Mrow2[ri:ri + 1, :].rearrange("p (l s) -> p l s", l=NL)[:, :, 0:NS],
                BT[qi * 5:qi * 5 + 5, :])
        rowip = sml.tile([1, SL2], F32, name="rowiP")
        nc.gpsimd.memset(rowip[:], 0.0)
        nc.sync.dma_start(rowip[:].rearrange("p (l s) -> p l s", l=NL)[:, :, 0:NS],
                          BT[25:30, :])
        bciP = med.tile([128, SL2], F32)
        nc.gpsimd.partition_broadcast(bciP[:], rowip[:], channels=128)

        # ================= roots [128, SL2] =================
        psarg = ps1.tile([128, SL2], F32, tag="psarg")
        nc.tensor.matmul(psarg[:], Y3[:], Hrow3[:], start=True, stop=True)
        valid = med.tile([128, SL2], F32)
        nc.vector.tensor_scalar(valid[:], psarg[:], 0.0, None, Alu.is_ge)
        arg = med.tile([128, SL2], F32)
        nc.vector.tensor_scalar(arg[:], psarg[:], 0.0, None, Alu.max)
        rt = med.tile([128, SL2], F32)
        nc.scalar.sqrt(rt[:], arg[:])
        rrec = med.tile([128, SL2], F32, tag="tq2")
        nc.vector.tensor_scalar(rrec[:], rt[:], 1e-30, None, Alu.max)
        nc.vector.reciprocal(rrec[:], rrec[:])
        nc.vector.tensor_tensor(out=rrec[:], in0=arg[:], in1=rrec[:], op=Alu.mult)
        nc.vector.tensor_tensor(out=rt[:], in0=rt[:], in1=rrec[:], op=Alu.add)
        nc.vector.tensor_scalar(rt[:], rt[:], 0.5, None, Alu.mult)
        half = rt
        nc.vector.tensor_tensor(out=half[:], in0=rt[:], in1=bciP[:], op=Alu.mult)
        psmrow = ps1.tile([128, SL2], F32, tag="psarg")
        nc.tensor.matmul(psmrow[:], Y2[:], Mrow2[:], start=True, stop=True)
        xlo = med.tile([128, SL2], F32)
        nc.vector.tensor_tensor(out=xlo[:], in0=psmrow[:], in1=half[:], op=Alu.subtract)
        nc.vector.tensor_scalar(xlo[:], xlo[:], 0.0, 127.0, Alu.max, Alu.min)
        xhi = med.tile([128, SL2], F32)
        nc.vector.tensor_tensor(out=xhi[:], in0=psmrow[:], in1=half[:], op=Alu.add)
        nc.vector.tensor_scalar(xhi[:], xhi[:], 0.0, 127.0, Alu.max, Alu.min)
        # nhi = floor(xhi), nlo = ceil(xlo) via int truncation
        nint = med.tile([128, SL2], I32, tag="nint")
        nc.vector.tensor_copy(nint[:], xhi[:])
        nhi = med.tile([128, SL2], F32)
        nc.vector.tensor_copy(nhi[:], nint[:])
        fhi = med.tile([128, SL2], F32, tag="tq3")
        nc.vector.tensor_tensor(out=fhi[:], in0=nhi[:], in1=xhi[:], op=Alu.is_gt)
        nc.vector.tensor_tensor(out=nhi[:], in0=nhi[:], in1=fhi[:], op=Alu.subtract)
        nc.vector.tensor_copy(nint[:], xlo[:])
        nlo = med.tile([128, SL2], F32)
        nc.vector.tensor_copy(nlo[:], nint[:])
        frac = med.tile([128, SL2], F32, tag="tq3")
        nc.vector.tensor_tensor(out=frac[:], in0=xlo[:], in1=nlo[:], op=Alu.is_gt)
        nc.vector.tensor_tensor(out=nlo[:], in0=nlo[:], in1=frac[:], op=Alu.add)
        cnt = med.tile([128, SL2], F32)
        nc.vector.tensor_tensor(out=cnt[:], in0=nhi[:], in1=nlo[:], op=Alu.subtract)
        nc.vector.tensor_scalar(cnt[:], cnt[:], 1.0, 0.0, Alu.add, Alu.max)
        nc.vector.tensor_tensor(out=cnt[:], in0=cnt[:], in1=valid[:], op=Alu.mult)
        okm = med.tile([128, SL2], F32, tag="tq")
        nc.vector.tensor_scalar(okm[:], cnt[:], 0.5, None, Alu.is_ge)

        # markers: idx = ok ? (n + 130*sl) : -1, interleaved (lo, hi) pairs
        M16 = med.tile([128, SL2 * 2], I16)
        m16v = M16[:].rearrange("p (q two) -> p q two", two=2)
        tmod = med.tile([128, SL2], F32, tag="tq2")
        nc.vector.tensor_tensor(out=tmod[:], in0=nlo[:], in1=LB1[:], op=Alu.add)
        nc.vector.tensor_tensor(out=tmod[:], in0=tmod[:], in1=okm[:], op=Alu.mult)
        nc.vector.tensor_scalar(tmod[:], tmod[:], 1.0, None, Alu.subtract)
        nc.vector.tensor_copy(m16v[:, :, 0], tmod[:])
        nc.vector.tensor_scalar(nhi[:], nhi[:], 1.0, None, Alu.add)
        nc.vector.tensor_tensor(out=tmod[:], in0=nhi[:], in1=LB1[:], op=Alu.add)
        nc.vector.tensor_tensor(out=tmod[:], in0=tmod[:], in1=okm[:], op=Alu.mult)
        nc.vector.tensor_scalar(tmod[:], tmod[:], 1.0, None, Alu.subtract)
        nc.vector.tensor_copy(m16v[:, :, 1], tmod[:])

        # ================= marker scatter + I1 =================
        # scatter +-1 markers, multiply by P2 prefix in place, fold segment
        # halves (130 -> 65) in bf16, then reduce the folded half-segments.
        red = med.tile([128, SL2], F32, name="red")
        fold = big.tile([128, NSLOT * 65], BF16, name="fold", tag="bigA")
        for l in range(NL):
            D = dpool.tile([128, DW], BF16, name=f"D{l}", tag="dtile")
            for w in range(NW):
                base = (l * NW + w) * (SPW * 2)
                nc.gpsimd.local_scatter(
                    D[:, w * WELEM:(w + 1) * WELEM],
                    data26[:],
                    M16[:, base:base + SPW * 2],
                    channels=128, num_elems=WELEM, num_idxs=SPW * 2)
            nc.vector.tensor_tensor(out=D[:], in0=D[:], in1=P2[:], op=Alu.mult)
            Dv = D[:].rearrange("p (s c) -> p s c", s=NSLOT)
            fv65 = fold[:].rearrange("p (s c) -> p s c", s=NSLOT)
            nc.vector.tensor_tensor(out=fv65, in0=Dv[:, :, 0:65],
                                    in1=Dv[:, :, 65:130], op=Alu.add)
            nc.vector.tensor_reduce(
                red[:, l * NSLOT:(l + 1) * NSLOT], fv65, AX.X, Alu.add)

        psI1 = ps1.tile([1, SL2], F32, tag="psI1")
        nc.tensor.matmul(psI1[:], onescol[:], red[:], start=True, stop=True)
        psI0 = ps1.tile([1, SL2], F32, tag="psb")
        nc.tensor.matmul(psI0[:], onescol[:], cnt[:], start=True, stop=True)

        # ================= metric + argmax (on [1, SL] (l,64) layout) ==============
        SL = NL * NS
        I1r = sml.tile([1, SL], F32)
        nc.vector.tensor_copy(I1r[:].rearrange("p (l s) -> p l s", l=NL),
                              psI1[:].rearrange("p (l s) -> p l s", l=NL)[:, :, 0:NS])
        I0r = sml.tile([1, SL], F32)
        nc.vector.tensor_copy(I0r[:].rearrange("p (l s) -> p l s", l=NL),
                              psI0[:].rearrange("p (l s) -> p l s", l=NL)[:, :, 0:NS])
        iin = sml.tile([1, SL], F32)
        nc.vector.tensor_scalar(iin[:], I0r[:], 0.5, None, Alu.mult)
        nc.vector.tensor_tensor(out=iin[:], in0=iin[:], in1=I1r[:], op=Alu.add)
        met = sml.tile([1, SL], F32)
        nc.vector.tensor_tensor(out=met[:].rearrange("p (l s) -> p s l", l=NL),
                                in0=iin[:].rearrange("p (l s) -> p s l", l=NL),
                                in1=itr[:].to_broadcast((1, NS, NL)), op=Alu.mult)
        nc.vector.tensor_scalar(I0r[:], I0r[:], float(1.0 / NPIX), None, Alu.mult)
        nc.vector.tensor_tensor(out=met[:], in0=met[:], in1=I0r[:], op=Alu.subtract)
        mmax = sml.tile([1, NS], F32)
        nc.vector.tensor_reduce(mmax[:], met[:].rearrange("p (l s) -> p s l", l=NL), AX.X, Alu.max)
        lidx_i = sml.tile([1, SL], I32)
        nc.gpsimd.iota(lidx_i[:].rearrange("p (l s) -> p l s", l=NL),
                       pattern=[[1, NL], [0, NS]], base=0, channel_multiplier=0)
        cand = sml.tile([1, SL], F32)
        nc.vector.tensor_copy(cand[:], lidx_i[:])
        eqmax = sml.tile([1, SL], F32)
        nc.vector.tensor_tensor(out=eqmax[:].rearrange("p (l s) -> p s l", l=NL),
                                in0=met[:].rearrange("p (l s) -> p s l", l=NL),
                                in1=mmax[:].to_broadcast((1, NS, NL)), op=Alu.is_lt)
        nc.vector.tensor_scalar(eqmax[:], eqmax[:], 99.0, None, Alu.mult)
        nc.vector.tensor_tensor(out=cand[:], in0=cand[:], in1=eqmax[:], op=Alu.add)
        bestr = sml.tile([1, NS], F32)
        nc.vector.tensor_reduce(bestr[:], cand[:].rearrange("p (l s) -> p s l", l=NL), AX.X, Alu.min)

        # ================= output side =================
        lvlfr = sml.tile([1, NS], F32)
        nc.vector.tensor_scalar(lvlfr[:], bestr[:], 0.1, 0.3, Alu.mult, Alu.add)
        lvlfb = med.tile([128, NS], F32)
        nc.gpsimd.partition_broadcast(lvlfb[:], lvlfr[:], channels=128)
        taub = med.tile([128, NS], F32)
        nc.vector.tensor_tensor(out=taub[:], in0=lvlfb[:], in1=rngbO[:], op=Alu.mult)
        nc.vector.tensor_tensor(out=taub[:], in0=taub[:], in1=mnbO[:], op=Alu.add)

        # d = vo - taub (f32), then fo = Relu(d), go = Sign(d) on the Act engine
        dfield = big.tile([128, F], F32, name="dfield", tag="bigA")
        nc.vector.tensor_tensor(out=dfield[:].rearrange("p (s x) -> p s x", s=NS),
                                in0=vo[:].rearrange("p (s x) -> p s x", s=NS),
                                in1=taub[:].to_broadcast((128, NS, W)), op=Alu.subtract)
        # f32 warmup matmuls on dfield ramp the PE pstate before the Mo moments
        pswarm = ps1.tile([1, 512], F32, tag="psI1")
        for wi in range(2):
            nc.tensor.matmul(pswarm[:], onescol[:], dfield[:, wi * 512:(wi + 1) * 512],
                             start=True, stop=True)
        fo = dpool.tile([128, DW], BF16, name="fo", tag="dtile")
        nc.scalar.activation(fo[:, 0:F], dfield[:], Act.Relu, bias=0.0, scale=1.0)
        go = dpool.tile([128, DW], BF16, name="go", tag="dtile")
        nc.vector.tensor_scalar(go[:, 0:F], dfield[:], 0.0, None, Alu.is_ge)

        SB = med.tile([NS, 12], F32)
        moments(fo, 100, SB, 0)
        moments(go, 101, SB, 6)

        def row2col(rowap, nm):
            pr = ps1.tile([NS, 1], F32, name=f"pr{nm}", tag="psmisc")
            nc.tensor.transpose(pr[:], rowap, eye128[0:1, 0:1])
            c = med.tile([NS, 1], F32, name=f"col{nm}")
            nc.vector.tensor_copy(c[:], pr[:])
            return c

        bestc = row2col(bestr[:], "best")
        mnoc = mncO
        rngoc = rngcO

        def col(nm):
            return med.tile([NS, 1], F32, name=nm)

        lvfc = col("lvfc")
        nc.vector.tensor_scalar(lvfc[:], bestc[:], 0.1, 0.3, Alu.mult, Alu.add)
        tauc = col("tauc")
        nc.vector.tensor_tensor(out=tauc[:], in0=lvfc[:], in1=rngoc[:], op=Alu.mult)
        nc.vector.tensor_tensor(out=tauc[:], in0=tauc[:], in1=mnoc[:], op=Alu.add)
        tmn = col("tmn")
        nc.vector.tensor_tensor(out=tmn[:], in0=tauc[:], in1=mnoc[:], op=Alu.subtract)
        WallO = med.tile([NS, 6], F32, name="WallO")
        nc.vector.tensor_scalar(WallO[:], SB[:, 6:12], tmn[:], None, Alu.mult)
        nc.vector.tensor_tensor(out=WallO[:], in0=WallO[:], in1=SB[:, 0:6], op=Alu.add)
        WmO = {j: WallO[:, j:j + 1] for j in range(6)}

        po = fit_params(WmO, "po", 1, False, refine=False)

        # select target params at best level
        eqm = med.tile([NS, NL], F32, name="eqm")
        l5f = med.tile([NS, NL], F32, name="l5f")
        nc.vector.tensor_copy(l5f[:], lr_i[:])
        nc.vector.tensor_scalar(eqm[:], l5f[:], bestc[:], None, Alu.is_equal)

        def select(src, nm):
            o = med.tile([NS, 1], F32, name="sel" + nm)
            tmp = med.tile([NS, NL], F32, name="selt" + nm, tag="seltmp")
            nc.vector.tensor_tensor(out=tmp[:], in0=src[:], in1=eqm[:], op=Alu.mult)
            nc.vector.tensor_reduce(o[:], tmp[:], AX.X, Alu.add)
            return o

        cxT = select(pt["cx"], "cx"); cyT = select(pt["cy"], "cy")
        cthT = select(pt["cth"], "ct"); sthT = select(pt["sth"], "st")
        aT = select(pt["a"], "a"); bT = select(pt["b"], "b")

        # ================= sym loss =================
        sc = col("sc")
        nc.vector.tensor_tensor(out=sc[:], in0=po["a"][:], in1=po["b"][:], op=Alu.max)
        t1c = col("t1c")
        nc.vector.tensor_tensor(out=t1c[:], in0=aT[:], in1=bT[:], op=Alu.max)
        nc.vector.tensor_tensor(out=sc[:], in0=sc[:], in1=t1c[:], op=Alu.max)
        nc.vector.tensor_scalar(sc[:], sc[:], float(EPS), None, Alu.add)
        isc = col("isc")
        nc.vector.reciprocal(isc[:], sc[:])
        lossc = col("lossc")
        td = col("td")

        def sqdiff_acc(xo, xt, first=False):
            nc.vector.tensor_tensor(out=td[:], in0=xo, in1=xt, op=Alu.subtract)
            nc.vector.tensor_tensor(out=td[:], in0=td[:], in1=isc[:], op=Alu.mult)
            nc.vector.tensor_tensor(out=td[:], in0=td[:], in1=td[:], op=Alu.mult)
            if first:
                nc.vector.tensor_copy(lossc[:], td[:])
            else:
                nc.vector.tensor_tensor(out=lossc[:], in0=lossc[:], in1=td[:], op=Alu.add)

        sqdiff_acc(po["cx"][:], cxT[:], first=True)
        sqdiff_acc(po["cy"][:], cyT[:])
        sqdiff_acc(po["a"][:], aT[:])
        sqdiff_acc(po["b"][:], bT[:])
        nc.vector.tensor_scalar(lossc[:], lossc[:], 0.5, None, Alu.mult)
        csum = col("csum")
        nc.vector.tensor_tensor(out=csum[:], in0=po["cth"][:], in1=cthT[:], op=Alu.mult)
        nc.vector.tensor_tensor(out=td[:], in0=po["sth"][:], in1=sthT[:], op=Alu.mult)
        nc.vector.tensor_tensor(out=csum[:], in0=csum[:], in1=td[:], op=Alu.add)
        nc.vector.tensor_scalar(csum[:], csum[:], -1.0, 1.0, Alu.mult, Alu.add)
        nc.vector.tensor_tensor(out=lossc[:], in0=lossc[:], in1=csum[:], op=Alu.add)

        nc.sync.dma_start(loss_out[:, :], lossc[:])


def build(NS=64, num_devices=1):
    nc = bacc.Bacc("TRN2", target_bir_lowering=False, debug=False, num_devices=num_devices)
    with tile.TileContext(nc) as tc:
        emit(nc, tc, NS=NS)
    nc.compile()
    return nc


_CACHED = {}


def _get_nc():
    if "nc" not in _CACHED:
        _CACHED["nc"] = build(NS=64, num_devices=8)
    return _CACHED["nc"]


def _make_in_maps(output, target, n_cores=8):
    output = np.ascontiguousarray(output, dtype=np.float32)
    target = np.ascontiguousarray(target, dtype=np.float32)
    per = output.shape[0] // n_cores
    in_maps = []
    for c in range(n_cores):
        sl = slice(c * per, (c + 1) * per)
        in_maps.append({
            "t": np.ascontiguousarray(target[sl].transpose(1, 0, 2)),
            "o": np.ascontiguousarray(output[sl].transpose(1, 0, 2)),
        })
    return in_maps


def kernel(output, target):
    from concourse.bass_utils import run_bass_kernel_spmd

    nc = _get_nc()
    in_maps = _make_in_maps(output, target)
    res = run_bass_kernel_spmd(nc, in_maps, core_ids=list(range(8)))
    losses = np.concatenate([r["loss"].reshape(-1) for r in res.results])
    return np.float32(losses.mean(dtype=np.float64))


# revision 29
# speedup vs baseline: 1.1552x; 1.0271x over previous
"""Trainium2 Bass kernel v3 for nn_BestEllipseLoss_5720896438361.

v3 over v2: fields moved to ACT engine (Relu/Sign with constant bias from a
bf16 normalized image), moment matmuls regrouped to 16x(N=512, M=96),
interval-sum reduce split into a bf16 halving fold + smaller f32-out reduce,
P2 memset shrunk to segment boundary columns, pure-f32 output-side dfield.

kernel(output, target): full [512,128,128] f32 inputs -> scalar f32 loss.
Shards batch across 8 NeuronCores (64 samples each), one SPMD Bass kernel.
"""
import sys
if "/opt/trn_rl_repo" not in sys.path:
    sys.path.insert(0, "/opt/trn_rl_repo")

import numpy as np

import concourse.bass as bass
import concourse.bacc as bacc
import concourse.tile as tile
import concourse.mybir as mybir
import concourse.bass_isa as bass_isa
import bass_rust

F32 = mybir.dt.float32
BF16 = mybir.dt.bfloat16
I32 = mybir.dt.int32
I16 = mybir.dt.int16
Alu = mybir.AluOpType
Act = mybir.ActivationFunctionType
AX = mybir.AxisListType

EPS = np.float32(1e-8)
LEVELS = [np.float32(0.3), np.float32(0.4), np.float32(0.5), np.float32(0.6), np.float32(0.7)]
NL = 5
H = 128
W = 128
NPIX = float(H * W)

_x = np.arange(W, dtype=np.float64)
_y = np.arange(H, dtype=np.float64)
# basis order j: {1, y, y^2, x, x*y, x^2}
C_B = np.array([
    H * W, W * _y.sum(), W * (_y ** 2).sum(),
    H * _x.sum(), _x.sum() * _y.sum(), H * (_x ** 2).sum(),
], dtype=np.float64).astype(np.float32)

# marker-scatter geometry
SEG = 130            # per-sample segment width in D / P2
SPW = 13             # samples per scatter window (SPW*SEG <= 2046)
NW = 5               # windows
NSLOT = SPW * NW     # 65 slots (64 real + 1 dummy)
DW = NSLOT * SEG     # 8450
WELEM = SPW * SEG    # 1690 elements per scatter call

# moment matmul grouping: x-chunk 8, basis j=12 (hi/lo), M=96, N=512
XC = 8               # x positions per group
NG = W // XC         # 16 groups
MROW = XC * 12       # 96 stationary rows


def _scan3d(eng, out, data0, data1, initial, op0, op1):
    """tensor_tensor_scan with multi-free-dim APs (recurrence chains across
    slices; data0 mask handles segment resets)."""
    return eng.add_instruction(
        mybir.InstTensorScalarPtr(
            name=eng.bass.get_next_instruction_name(),
            is_tensor_tensor_scan=True,
            is_scalar_tensor_tensor=True,
            op0=op0,
            op1=op1,
            ins=[
                eng.lower_ap(data0),
                eng.lower_ap_or_imm(initial),
                eng.lower_ap(data1),
            ],
            outs=[eng.lower_ap(out)],
        )
    )


def _ovl(ap, nslot, seg, stride):
    """Overlapping windowed view [(p), nslot, seg] with window stride."""
    v = ap.copy()
    v.ap = bass_rust.VecI64Pair([list(ap.ap[0]), [stride, nslot], [1, seg]])
    return v


def emit(nc, tc, NS=64):
    F = NS * W
    SL2 = NL * NSLOT      # 325, (l-major, 65-slot) layout

    t_in = nc.dram_tensor("t", [H, NS, W], F32, kind="ExternalInput")
    o_in = nc.dram_tensor("o", [H, NS, W], F32, kind="ExternalInput")
    loss_out = nc.dram_tensor("loss", [NS, 1], F32, kind="ExternalOutput")

    with tc.tile_pool(name="big", bufs=1) as big, \
         tc.tile_pool(name="dpool", bufs=2) as dpool, \
         tc.tile_pool(name="fg", bufs=1) as fg, \
         tc.tile_pool(name="med", bufs=1) as med, \
         tc.tile_pool(name="sml", bufs=1) as sml, \
         tc.tile_pool(name="ps", bufs=2, space="PSUM") as ps, \
         tc.tile_pool(name="ps1", bufs=1, space="PSUM") as ps1:

        # ================= constants =================
        yi = sml.tile([128, 1], I32)
        nc.gpsimd.iota(yi[:], pattern=[[0, 1]], base=0, channel_multiplier=1)
        yv = sml.tile([128, 1], F32)
        nc.vector.tensor_copy(yv[:], yi[:])
        y2v = sml.tile([128, 1], F32)
        nc.vector.tensor_tensor(out=y2v[:], in0=yv[:], in1=yv[:], op=Alu.mult)

        ei = med.tile([128, 128], I32, tag="scrA")
        nc.gpsimd.iota(ei[:], pattern=[[1, 128]], base=0, channel_multiplier=-1)
        eif = med.tile([128, 128], F32, tag="scrB")
        nc.vector.tensor_copy(eif[:], ei[:])
        eye128 = med.tile([128, 128], F32)
        nc.vector.tensor_scalar(eye128[:], eif[:], 0.0, None, Alu.is_equal)

        onescol = sml.tile([128, 1], F32)
        nc.gpsimd.memset(onescol[:], 1.0)
        onescolb = sml.tile([128, 1], BF16)
        nc.gpsimd.memset(onescolb[:], 1.0)

        lvl_bias = []
        for l in range(NL):
            b = sml.tile([128, 1], F32, name=f"lvlb{l}")
            nc.gpsimd.memset(b[:], -float(LEVELS[l] - np.float32(0.5)))
            lvl_bias.append(b)

        # SEL_i [96, 6] fold matrices: pick q = 12*i + j (hi) and 12*i+6+j (lo)
        di = med.tile([MROW, 6], I32, tag="scrC")
        nc.gpsimd.iota(di[:], pattern=[[-1, 6]], base=0, channel_multiplier=1)
        df = med.tile([MROW, 6], F32, tag="scrD")
        nc.vector.tensor_copy(df[:], di[:])
        SELS = []
        for i in range(XC):
            s1 = med.tile([MROW, 6], F32, name=f"sel{i}")
            nc.vector.tensor_scalar(s1[:], df[:], float(12 * i), None, Alu.is_equal)
            s2 = med.tile([MROW, 6], F32, name=f"sel2_{i}", tag="scrE")
            nc.vector.tensor_scalar(s2[:], df[:], float(12 * i + 6), None, Alu.is_equal)
            nc.vector.tensor_tensor(out=s1[:], in0=s1[:], in1=s2[:], op=Alu.add)
            SELS.append(s1)

        # ---- moment lhsT table [128, NG*MROW] bf16 (hi/lo split basis) ----
        mast = med.tile([128, 768], F32, tag="scrA")
        nc.gpsimd.memset(mast[:], 1.0)
        xri = sml.tile([1, W], I32)
        nc.gpsimd.iota(xri[:], pattern=[[1, W]], base=0, channel_multiplier=0)
        xrf = sml.tile([1, W], F32)
        nc.vector.tensor_copy(xrf[:], xri[:])
        x2rf = sml.tile([1, W], F32)
        nc.vector.tensor_tensor(out=x2rf[:], in0=xrf[:], in1=xrf[:], op=Alu.mult)
        xfull = med.tile([128, W], F32, tag="scrB")
        nc.gpsimd.partition_broadcast(xfull[:], xrf[:], channels=128)
        x2full = med.tile([128, W], F32, tag="scrF")
        nc.gpsimd.partition_broadcast(x2full[:], x2rf[:], channels=128)
        mv = mast[:].rearrange("p (g t j) -> p g t j", g=NG, t=XC)
        xfv = xfull[:].rearrange("p (g t) -> p g t", g=NG)
        x2fv = x2full[:].rearrange("p (g t) -> p g t", g=NG)
        nc.vector.tensor_copy(mv[:, :, :, 3:4], xfv.to_broadcast((128, NG, XC, 1)))
        nc.vector.tensor_copy(mv[:, :, :, 4:5], xfv.to_broadcast((128, NG, XC, 1)))
        nc.vector.tensor_copy(mv[:, :, :, 5:6], x2fv.to_broadcast((128, NG, XC, 1)))
        mgt = mast[:].rearrange("p (gt j) -> p gt j", j=6)
        nc.vector.tensor_scalar(mgt[:, :, 1:2], mgt[:, :, 1:2], yv[:], None, Alu.mult)
        nc.vector.tensor_scalar(mgt[:, :, 4:5], mgt[:, :, 4:5], yv[:], None, Alu.mult)
        nc.vector.tensor_scalar(mgt[:, :, 2:3], mgt[:, :, 2:3], y2v[:], None, Alu.mult)
        hi24 = med.tile([128, 768], BF16, tag="scrG")
        nc.vector.tensor_copy(hi24[:], mast[:])
        lo24 = mast
        nc.vector.tensor_tensor(out=lo24[:], in0=mast[:], in1=hi24[:], op=Alu.subtract)
        table = med.tile([128, NG * MROW], BF16)
        tvv = table[:].rearrange("p (g t j) -> p g t j", g=NG, t=XC)
        nc.vector.tensor_copy(tvv[:, :, :, 0:6], hi24[:].rearrange("p (g t j) -> p g t j", g=NG, t=XC))
        nc.vector.tensor_copy(tvv[:, :, :, 6:12], lo24[:].rearrange("p (g t j) -> p g t j", g=NG, t=XC))

        # lvl rows [NS, NL]
        lr_i = sml.tile([NS, NL], I32)
        nc.gpsimd.iota(lr_i[:], pattern=[[1, NL]], base=0, channel_multiplier=0)

        # marker bases: LB1 = 130*sl + 1 over (l, w, sl)
        lb_i = med.tile([128, SL2], I32, tag="nint")
        nc.gpsimd.iota(lb_i[:].rearrange("p (l w s) -> p l w s", l=NL, w=NW),
                       pattern=[[0, NL], [0, NW], [SEG, SPW]], base=1,
                       channel_multiplier=0)
        LB1 = med.tile([128, SL2], F32)
        nc.vector.tensor_copy(LB1[:], lb_i[:])
        # rank-2 lhsT for arg/mrow matmuls: Y3 = [y^2; y; 1], Y2 = [1; -y]
        y3c = med.tile([128, 3], F32, tag="scrD2")
        nc.vector.tensor_copy(y3c[:, 0:1], y2v[:])
        nc.vector.tensor_copy(y3c[:, 1:2], yv[:])
        nc.vector.tensor_scalar(y3c[:, 2:3], yv[:], 0.0, 1.0, Alu.mult, Alu.add)
        psy = ps1.tile([3, 128], F32, tag="psmisc")
        nc.tensor.transpose(psy[:], y3c[:], eye128[:, :])
        Y3 = med.tile([3, 128], F32)
        nc.vector.tensor_copy(Y3[:], psy[:])
        y2c = med.tile([128, 2], F32, tag="scrD3")
        nc.vector.tensor_scalar(y2c[:, 0:1], yv[:], 0.0, 1.0, Alu.mult, Alu.add)
        nc.vector.tensor_scalar(y2c[:, 1:2], yv[:], -1.0, None, Alu.mult)
        psy2 = ps1.tile([2, 128], F32, tag="psmisc")
        nc.tensor.transpose(psy2[:], y2c[:], eye128[:, :])
        Y2 = med.tile([2, 128], F32)
        nc.vector.tensor_copy(Y2[:], psy2[:])

        # scatter data row: (-1, +1) x SPW in bf16
        d26i = sml.tile([128, SPW * 2], I32)
        nc.gpsimd.iota(d26i[:], pattern=[[0, SPW], [1, 2]], base=0, channel_multiplier=0)
        d26f = sml.tile([128, SPW * 2], F32)
        nc.vector.tensor_copy(d26f[:], d26i[:])
        data26 = sml.tile([128, SPW * 2], BF16)
        nc.vector.tensor_scalar(data26[:], d26f[:], 2.0, -1.0, Alu.mult, Alu.add)

        # ================= loads =================
        hp = tc.high_priority(offset=100000)
        hp.__enter__()
        # 4-queue split load: each engine's DGE processes its descriptor batch
        # concurrently, ~4x the single-queue descriptor rate
        vt = big.tile([128, F], F32, tag="bigA")
        tv_flat = t_in[:, :, :].rearrange("y s x -> y (s x)")
        tengs = (nc.sync, nc.scalar, nc.gpsimd, nc.sync)
        for i in range(4):
            tengs[i].dma_start(vt[:, i * (F // 4):(i + 1) * (F // 4)],
                               tv_flat[:, i * (F // 4):(i + 1) * (F // 4)])
        vo = big.tile([128, F], F32, tag="bigO", name="vo")
        ov_flat = o_in[:, :, :].rearrange("y s x -> y (s x)")
        for i in range(2):
            eng = (nc.scalar, nc.gpsimd)[i % 2]
            eng.dma_start(vo[:, i * (F // 2):(i + 1) * (F // 2)],
                          ov_flat[:, i * (F // 2):(i + 1) * (F // 2)])

        # ================= min/max + normalize (target) =================
        # per-partition max / negated min land side by side in mmT; one PE
        # transpose + free-dim reduce replaces gpsimd partition_all_reduce
        mmT = med.tile([128, 2 * NS], F32, name="mmT", tag="mxp")
        NH2 = NS // 2
        NQ = NS // 4
        for q in range(4):
            hv = vt[:, q * NQ * W:(q + 1) * NQ * W].rearrange("p (s x) -> p s x", s=NQ)
            nc.vector.tensor_reduce(mmT[:, q * NQ:(q + 1) * NQ], hv, AX.X, Alu.max)
            nc.vector.tensor_reduce(mmT[:, NS + q * NQ:NS + (q + 1) * NQ], hv, AX.X, Alu.min)
        nc.vector.tensor_scalar(mmT[:, NS:2 * NS], mmT[:, NS:2 * NS], -1.0, None, Alu.mult)
        # separate transposes rebase both halves to partition 0
        psmxT = ps1.tile([NS, 128], F32, tag="psarg")
        nc.tensor.transpose(psmxT[:], mmT[:, 0:NS], eye128[:, :])
        mxcT = med.tile([NS, 1], F32, name="mxcT")
        nc.vector.tensor_reduce(mxcT[:], psmxT[:], AX.X, Alu.max)
        psmnT = ps1.tile([NS, 128], F32, tag="psarg")
        nc.tensor.transpose(psmnT[:], mmT[:, NS:2 * NS], eye128[:, :])
        negmnT = med.tile([NS, 1], F32, name="negmnT")
        nc.vector.tensor_reduce(negmnT[:], psmnT[:], AX.X, Alu.max)
        rngcT = med.tile([NS, 1], F32, name="rngcT")
        nc.vector.tensor_tensor(out=rngcT[:], in0=mxcT[:], in1=negmnT[:], op=Alu.add)
        nc.vector.tensor_scalar(rngcT[:], rngcT[:], float(EPS), None, Alu.add)
        rbcT = med.tile([NS, 1], F32, name="rbcT")
        nc.vector.reciprocal(rbcT[:], rngcT[:])
        shcT = med.tile([NS, 1], F32, name="shcT")
        nc.vector.tensor_scalar(shcT[:], rngcT[:], 0.5, None, Alu.mult)
        nc.vector.tensor_tensor(out=shcT[:], in0=shcT[:], in1=negmnT[:], op=Alu.subtract)
        # rowify (shift, rb) -> [128, NS] broadcast rows
        def rowify(colap, bcast_out, nm):
            pr = ps1.tile([1, NS], F32, tag="psmisc", name=f"psr{nm}")
            nc.tensor.transpose(pr[:], colap, eye128[0:NS, 0:NS])
            r = med.tile([1, NS], F32, name=f"row{nm}")
            nc.vector.tensor_copy(r[:], pr[:])
            nc.gpsimd.partition_broadcast(bcast_out, r[:], channels=128)
            return r

        shiftT = med.tile([128, NS], F32)
        rowify(shcT[:], shiftT[:], "shT")
        rbT = med.tile([128, NS], F32)
        rowify(rbcT[:], rbT[:], "rbT")

        for hi2 in range(2):
            upv = vt[:, hi2 * NH2 * W:(hi2 + 1) * NH2 * W].rearrange("p (s x) -> p s x", s=NH2)
            sh = shiftT[:, hi2 * NH2:(hi2 + 1) * NH2]
            rb = rbT[:, hi2 * NH2:(hi2 + 1) * NH2]
            nc.vector.tensor_tensor(out=upv, in0=upv,
                                    in1=sh.to_broadcast((128, NH2, W)), op=Alu.subtract)
            nc.vector.tensor_tensor(out=upv, in0=upv,
                                    in1=rb.to_broadcast((128, NH2, W)), op=Alu.mult)
        # ================= fields + moments (target) =================
        SA = med.tile([NS, 72], F32)

        def moments_mm(field_tile, fl):
            psm = ps.tile([MROW, NS * XC], F32, name=f"psm{fl}", tag="psmom")
            fv = field_tile[:, 0:F].rearrange("p (s x) -> p s x", s=NS)
            for g in range(NG):
                nc.tensor.matmul(psm[:], table[:, g * MROW:(g + 1) * MROW],
                                 fv[:, :, XC * g:XC * g + XC],
                                 start=(g == 0), stop=(g == NG - 1))
            return psm

        def moments_fold(psm, fl, out_tile, col0):
            S96 = med.tile([MROW, NS * XC], F32, name=f"s96_{fl}", tag=f"s96{fl % 2}")
            nc.vector.tensor_copy(S96[:], psm[:])
            pT = ps.tile([NS, 6], F32, name=f"pT{fl}", tag="ps6")
            sv = S96[:].rearrange("q (s t) -> q s t", s=NS)
            for i in range(XC):
                nc.tensor.matmul(pT[:], sv[:, :, i], SELS[i][:],
                                 start=(i == 0), stop=(i == XC - 1))
            nc.vector.tensor_copy(out_tile[:, col0:col0 + 6], pT[:])

        def moments(field_tile, fl, out_tile, col0):
            moments_fold(moments_mm(field_tile, fl), fl, out_tile, col0)

        # deferred folds keep the PE matmul stream continuous (pstate ramp)
        for l in range(NL):
            lvlp = float(LEVELS[l] - np.float32(0.5))
            f_l = fg.tile([128, F], BF16, name=f"f{l}", tag="ffield")
            nc.vector.tensor_scalar(f_l[:], vt[:, 0:F], lvlp, lvlp, Alu.max, Alu.subtract)
            g_l = fg.tile([128, F], BF16, name=f"g{l}", tag="gfield")
            nc.scalar.activation(g_l[:], vt[:, 0:F], Act.Sign, bias=lvl_bias[l][:], scale=1.0)
            psf = moments_mm(f_l, l)
            psg = moments_mm(g_l, 5 + l)
            moments_fold(psf, l, SA, l * 6)
            moments_fold(psg, 5 + l, SA, 30 + l * 6)

        # ============ segmented prefix scan -> P2 (bf16, windowed) ============
        # state = (rowmask * state) + u' resets at each sample's x=0; output
        # lands at window slot x+1, so slot k = sum_{x<k} u' per sample.
        mi_ = sml.tile([128, W], I32, name="mi_")
        nc.gpsimd.iota(mi_[:], pattern=[[1, W]], base=0, channel_multiplier=0)
        mf_ = sml.tile([128, W], F32, name="mf_")
        nc.vector.tensor_copy(mf_[:], mi_[:])
        nc.vector.tensor_scalar(mf_[:], mf_[:], 0.5, None, Alu.is_ge)
        P2 = big.tile([128, DW], BF16, name="P2", tag="bigP")
        p2segs = P2[:].rearrange("p (s c) -> p s c", s=NSLOT)
        nc.gpsimd.memset(p2segs[:, :, 0:1], 0.0)
        nc.gpsimd.memset(p2segs[:, :, 129:130], 0.0)
        _scan3d(nc.vector,
                _ovl(P2[:, 1:], NS, W, SEG),
                _ovl(mf_[:, :], NS, W, 0),
                vt[:, 0:F].rearrange("p (s x) -> p s x", s=NS),
                0.0, Alu.mult, Alu.add)

        # itot from P2 column 128: itot(s) = sum_y P'[y, s, 128]
        p2col = P2[:].rearrange("p (s c) -> p s c", s=NSLOT)[:, :, 128]
        psb = ps1.tile([1, NSLOT], F32, tag="psb")
        nc.tensor.matmul(psb[:], onescolb[:], p2col, start=True, stop=True)
        brow = sml.tile([1, NSLOT], F32)
        nc.vector.tensor_copy(brow[:], psb[:])
        itot = sml.tile([1, NS], F32)
        nc.vector.tensor_scalar(itot[:], brow[:, 0:NS], 0.5 * NPIX, float(EPS), Alu.add, Alu.add)
        itr = sml.tile([1, NS], F32)
        nc.vector.reciprocal(itr[:], itot[:])

        hp.__exit__(None, None, None)

        # minmaxO per-half (slots into field-phase DVE gaps)
        # eighth-chunked so the scheduler can slot these into small DVE gaps
        mmO = med.tile([128, 2 * NS], F32, name="mmO", tag="mxp")
        NE = NS // 8
        for q in range(8):
            hv = vo[:, q * NE * W:(q + 1) * NE * W].rearrange("p (s x) -> p s x", s=NE)
            nc.vector.tensor_reduce(mmO[:, q * NE:(q + 1) * NE], hv, AX.X, Alu.max)
            nc.vector.tensor_reduce(mmO[:, NS + q * NE:NS + (q + 1) * NE], hv, AX.X, Alu.min)
        nc.vector.tensor_scalar(mmO[:, NS:2 * NS], mmO[:, NS:2 * NS], -1.0, None, Alu.mult)
        psmxO = ps1.tile([NS, 128], F32, tag="psarg")
        nc.tensor.transpose(psmxO[:], mmO[:, 0:NS], eye128[:, :])
        mxcO = med.tile([NS, 1], F32, name="mxcO")
        nc.vector.tensor_reduce(mxcO[:], psmxO[:], AX.X, Alu.max)
        psmnO = ps1.tile([NS, 128], F32, tag="psarg")
        nc.tensor.transpose(psmnO[:], mmO[:, NS:2 * NS], eye128[:, :])
        negmnO = med.tile([NS, 1], F32, name="negmnO")
        nc.vector.tensor_reduce(negmnO[:], psmnO[:], AX.X, Alu.max)
        # normalize-center vo in place: von = (vo - mn)/(rng+eps) - 0.5
        # (fit is scale-invariant, so all-5-level output moments can be
        # computed speculatively before the best level is known)
        rngpO = med.tile([NS, 1], F32, name="rngpO")
        nc.vector.tensor_tensor(out=rngpO[:], in0=mxcO[:], in1=negmnO[:], op=Alu.add)
        nc.vector.tensor_scalar(rngpO[:], rngpO[:], float(EPS), None, Alu.add)
        rbcO = med.tile([NS, 1], F32, name="rbcO")
        nc.vector.reciprocal(rbcO[:], rngpO[:])
        shcO = med.tile([NS, 1], F32, name="shcO")
        nc.vector.tensor_scalar(shcO[:], rngpO[:], 0.5, None, Alu.mult)
        nc.vector.tensor_tensor(out=shcO[:], in0=shcO[:], in1=negmnO[:], op=Alu.subtract)
        shbO = med.tile([128, NS], F32, name="shbO")
        rowify(shcO[:], shbO[:], "shO")
        rbbO = med.tile([128, NS], F32, name="rbbO")
        rowify(rbcO[:], rbbO[:], "rbO")
        for hi2 in range(2):
            ov = vo[:, hi2 * NH2 * W:(hi2 + 1) * NH2 * W].rearrange("p (s x) -> p s x", s=NH2)
            sh = shbO[:, hi2 * NH2:(hi2 + 1) * NH2]
            rb = rbbO[:, hi2 * NH2:(hi2 + 1) * NH2]
            nc.vector.tensor_tensor(out=ov, in0=ov,
                                    in1=sh.to_broadcast((128, NH2, W)), op=Alu.subtract)
            nc.vector.tensor_tensor(out=ov, in0=ov,
                                    in1=rb.to_broadcast((128, NH2, W)), op=Alu.mult)

        # ================= target params (sample layout [NS, NL]) =================
        def FA(j):
            return SA[:, j:j + 30:6]

        def GA(j):
            return SA[:, 30 + j:30 + j + 30:6]

        cbt = med.tile([NS, 30], F32, name="cbt")
        hlvl = med.tile([NS, 30], F32, name="hlvl")
        for j in range(6):
            nc.gpsimd.memset(cbt[:, j:30:6], float(C_B[j]))
        for l in range(NL):
            nc.gpsimd.memset(hlvl[:, l * 6:(l + 1) * 6], 0.5 * float(LEVELS[l]))
        Wall = med.tile([NS, 30], F32, name="Wall")
        nc.vector.tensor_tensor(out=Wall[:], in0=SA[:, 30:60], in1=cbt[:], op=Alu.add)
        nc.vector.tensor_tensor(out=Wall[:], in0=Wall[:], in1=hlvl[:], op=Alu.mult)
        nc.vector.tensor_tensor(out=Wall[:], in0=Wall[:], in1=SA[:, 0:30], op=Alu.add)
        Wm = {j: Wall[:, j:30:6] for j in range(6)}

        def fit_params(Wd, tagp, n_l, want_roots, refine=True):
            def nt(nm):
                return med.tile([NS, n_l], F32, name=tagp + nm)
            m00_ = nt("m00")
            nc.vector.tensor_scalar(m00_[:], Wd[0], float(EPS), None, Alu.add)
            im_ = nt("im")
            nc.vector.reciprocal(im_[:], m00_[:])
            cx_ = nt("cx"); cy_ = nt("cy"); tz = nt("tz")
            nc.vector.tensor_tensor(out=cx_[:], in0=Wd[3], in1=im_[:], op=Alu.mult)
            nc.vector.tensor_tensor(out=cy_[:], in0=Wd[1], in1=im_[:], op=Alu.mult)
            mu20_ = nt("mu20"); mu02_ = nt("mu02"); mu11_ = nt("mu11")
            nc.vector.tensor_tensor(out=mu20_[:], in0=Wd[5], in1=im_[:], op=Alu.mult)
            nc.vector.tensor_tensor(out=tz[:], in0=cx_[:], in1=cx_[:], op=Alu.mult)
            nc.vector.tensor_tensor(out=mu20_[:], in0=mu20_[:], in1=tz[:], op=Alu.subtract)
            nc.vector.tensor_tensor(out=mu02_[:], in0=Wd[2], in1=im_[:], op=Alu.mult)
            nc.vector.tensor_tensor(out=tz[:], in0=cy_[:], in1=cy_[:], op=Alu.mult)
            nc.vector.tensor_tensor(out=mu02_[:], in0=mu02_[:], in1=tz[:], op=Alu.subtract)
            nc.vector.tensor_tensor(out=mu11_[:], in0=Wd[4], in1=im_[:], op=Alu.mult)
            nc.vector.tensor_tensor(out=tz[:], in0=cx_[:], in1=cy_[:], op=Alu.mult)
            nc.vector.tensor_tensor(out=mu11_[:], in0=mu11_[:], in1=tz[:], op=Alu.subtract)
            dmu_ = nt("dmu"); smu_ = nt("smu"); cc_ = nt("cc")
            nc.vector.tensor_tensor(out=dmu_[:], in0=mu20_[:], in1=mu02_[:], op=Alu.subtract)
            nc.vector.tensor_tensor(out=smu_[:], in0=mu20_[:], in1=mu02_[:], op=Alu.add)
            nc.vector.tensor_tensor(out=cc_[:], in0=dmu_[:], in1=dmu_[:], op=Alu.mult)
            nc.vector.tensor_tensor(out=tz[:], in0=mu11_[:], in1=mu11_[:], op=Alu.mult)
            nc.vector.tensor_scalar(tz[:], tz[:], 4.0, None, Alu.mult)
            nc.vector.tensor_tensor(out=cc_[:], in0=cc_[:], in1=tz[:], op=Alu.add)
            com_ = nt("com")
            nc.scalar.sqrt(com_[:], cc_[:])
            gd = nt("gd"); rc = nt("rc")
            if refine:
                nc.vector.tensor_scalar(gd[:], com_[:], 1e-30, None, Alu.max)
                nc.vector.reciprocal(rc[:], gd[:])
                nc.vector.tensor_tensor(out=rc[:], in0=cc_[:], in1=rc[:], op=Alu.mult)
                nc.vector.tensor_tensor(out=com_[:], in0=com_[:], in1=rc[:], op=Alu.add)
                nc.vector.tensor_scalar(com_[:], com_[:], 0.5, None, Alu.mult)
            a2_ = nt("a2"); b2_ = nt("b2")
            nc.vector.tensor_tensor(out=a2_[:], in0=smu_[:], in1=com_[:], op=Alu.add)
            nc.vector.tensor_scalar(a2_[:], a2_[:], 2.0, float(EPS), Alu.mult, Alu.max)
            nc.vector.tensor_tensor(out=b2_[:], in0=smu_[:], in1=com_[:], op=Alu.subtract)
            nc.vector.tensor_scalar(b2_[:], b2_[:], 2.0, float(EPS), Alu.mult, Alu.max)
            a_ = nt("a"); b_ = nt("b")
            nc.scalar.sqrt(a_[:], a2_[:])
            nc.scalar.sqrt(b_[:], b2_[:])
            if refine:
                nc.vector.tensor_scalar(gd[:], a_[:], 1e-30, None, Alu.max)
                nc.vector.reciprocal(rc[:], gd[:])
                nc.vector.tensor_tensor(out=rc[:], in0=a2_[:], in1=rc[:], op=Alu.mult)
                nc.vector.tensor_tensor(out=a_[:], in0=a_[:], in1=rc[:], op=Alu.add)
                nc.vector.tensor_scalar(a_[:], a_[:], 0.5, None, Alu.mult)
                nc.vector.tensor_scalar(gd[:], b_[:], 1e-30, None, Alu.max)
                nc.vector.reciprocal(rc[:], gd[:])
                nc.vector.tensor_tensor(out=rc[:], in0=b2_[:], in1=rc[:], op=Alu.mult)
                nc.vector.tensor_tensor(out=b_[:], in0=b_[:], in1=rc[:], op=Alu.add)
                nc.vector.tensor_scalar(b_[:], b_[:], 0.5, None, Alu.mult)
            cg = nt("cg"); ic = nt("ic")
            nc.vector.tensor_scalar(cg[:], com_[:], 1e-30, None, Alu.max)
            nc.vector.reciprocal(ic[:], cg[:])
            cphi_ = nt("cphi"); sphi_ = nt("sphi")
            nc.vector.tensor_tensor(out=cphi_[:], in0=dmu_[:], in1=ic[:], op=Alu.mult)
            nc.vector.tensor_scalar(cphi_[:], cphi_[:], -1.0, 1.0, Alu.max, Alu.min)
            nc.vector.tensor_tensor(out=sphi_[:], in0=mu11_[:], in1=ic[:], op=Alu.mult)
            cth_ = nt("cth"); sth_ = nt("sth"); sg_ = nt("sg")
            nc.vector.tensor_scalar(cth_[:], cphi_[:], 1.0, 0.5, Alu.add, Alu.mult)
            nc.scalar.sqrt(cth_[:], cth_[:])
            nc.vector.tensor_scalar(sth_[:], cphi_[:], -1.0, None, Alu.mult)
            nc.vector.tensor_scalar(sth_[:], sth_[:], 1.0, 0.5, Alu.add, Alu.mult)
            nc.scalar.sqrt(sth_[:], sth_[:])
            nc.vector.tensor_scalar(sg_[:], sphi_[:], 0.0, None, Alu.is_ge)
            nc.vector.tensor_scalar(sg_[:], sg_[:], 2.0, -1.0, Alu.mult, Alu.add)
            nc.vector.tensor_tensor(out=sth_[:], in0=sth_[:], in1=sg_[:], op=Alu.mult)
            res = dict(cx=cx_, cy=cy_, cth=cth_, sth=sth_, a=a_, b=b_)
            if not want_roots:
                return res
            Aa = nt("Aa"); Bb = nt("Bb")
            nc.vector.tensor_scalar(Aa[:], a_[:], float(EPS), None, Alu.add)
            nc.vector.tensor_scalar(Bb[:], b_[:], float(EPS), None, Alu.add)
            iA2 = nt("iA2"); iB2 = nt("iB2")
            nc.vector.tensor_tensor(out=gd[:], in0=Aa[:], in1=Aa[:], op=Alu.mult)
            nc.vector.reciprocal(iA2[:], gd[:])
            nc.vector.tensor_tensor(out=gd[:], in0=Bb[:], in1=Bb[:], op=Alu.mult)
            nc.vector.reciprocal(iB2[:], gd[:])
            c2t = nt("c2t"); s2t = nt("s2t")
            nc.vector.tensor_tensor(out=c2t[:], in0=cth_[:], in1=cth_[:], op=Alu.mult)
            nc.vector.tensor_tensor(out=s2t[:], in0=sth_[:], in1=sth_[:], op=Alu.mult)
            Pq = nt("Pq")
            nc.vector.tensor_tensor(out=Pq[:], in0=c2t[:], in1=iA2[:], op=Alu.mult)
            nc.vector.tensor_tensor(out=tz[:], in0=s2t[:], in1=iB2[:], op=Alu.mult)
            nc.vector.tensor_tensor(out=Pq[:], in0=Pq[:], in1=tz[:], op=Alu.add)
            Rq = nt("Rq")
            nc.vector.tensor_tensor(out=Rq[:], in0=iA2[:], in1=iB2[:], op=Alu.subtract)
            nc.vector.tensor_tensor(out=Rq[:], in0=Rq[:], in1=cth_[:], op=Alu.mult)
            nc.vector.tensor_tensor(out=Rq[:], in0=Rq[:], in1=sth_[:], op=Alu.mult)
            K3 = nt("K3")
            nc.vector.tensor_tensor(out=K3[:], in0=iA2[:], in1=iB2[:], op=Alu.mult)
            iP = nt("iP")
            nc.vector.reciprocal(iP[:], Pq[:])
            K1 = nt("K1")
            nc.vector.tensor_tensor(out=K1[:], in0=Rq[:], in1=iP[:], op=Alu.mult)
            M0 = nt("M0")
            nc.vector.tensor_tensor(out=M0[:], in0=K1[:], in1=cy_[:], op=Alu.mult)
            nc.vector.tensor_tensor(out=M0[:], in0=M0[:], in1=cx_[:], op=Alu.add)
            H2 = nt("H2")
            nc.vector.tensor_scalar(H2[:], K3[:], -1.0, None, Alu.mult)
            H1 = nt("H1")
            nc.vector.tensor_tensor(out=H1[:], in0=K3[:], in1=cy_[:], op=Alu.mult)
            nc.vector.tensor_scalar(H1[:], H1[:], 2.0, None, Alu.mult)
            H0 = nt("H0")
            nc.vector.tensor_tensor(out=tz[:], in0=H1[:], in1=cy_[:], op=Alu.mult)
            nc.vector.tensor_scalar(tz[:], tz[:], 0.5, None, Alu.mult)
            nc.vector.tensor_tensor(out=H0[:], in0=Pq[:], in1=tz[:], op=Alu.subtract)
            res.update(M0=M0, K1=K1, H0=H0, H1=H1, H2=H2, iP=iP)
            return res

        pt = fit_params(Wm, "pt", NL, True)

        # ======== back-broadcast coef rows to [128, SL2] (l-major, 65-slot) ========
        BS = med.tile([NS, 30], F32)
        for qi, q in enumerate([pt["M0"], pt["K1"], pt["H0"], pt["H1"], pt["H2"], pt["iP"]]):
            nc.vector.tensor_copy(BS[:, qi * 5:qi * 5 + 5], q[:])
        psb2 = ps1.tile([30, NS], F32, tag="psmisc")
        nc.tensor.transpose(psb2[:], BS[:, :], eye128[0:NS, 0:NS])
        BT = med.tile([30, NS], F32)
        nc.vector.tensor_copy(BT[:], psb2[:])
        # benign dummy-slot values: arg = -1 -> invalid -> cnt 0 -> idx -1
        # BS col order: [M0, K1, H0, H1, H2, iP] -> BT rows qi*5+l
        # Hrow3 rows: H2, H1, H0 (dummies 0, 0, -1); Mrow2 rows: M0, K1 (dummies 0)
        # dummy slots stay 0 -> they produce a benign 1-px interval whose
        # contributions land only in dummy columns, sliced out downstream
        Hrow3 = med.tile([3, SL2], F32)
        nc.gpsimd.memset(Hrow3[:, :], 0.0)
        for ri, qi in ((0, 4), (1, 3), (2, 2)):
            nc.sync.dma_start(
                Hrow3[ri:ri + 1, :].rearrange("p (l s) -> p l s", l=NL)[:, :, 0:NS],
                BT[qi * 5:qi * 5 + 5, :])
        Mrow2 = med.tile([2, SL2], F32)
        nc.gpsimd.memset(Mrow2[:, :], 0.0)
        for ri, qi in ((0, 0), (1, 1)):
            nc.sync.dma_start(
                Mrow2[ri:ri + 1, :].rearrange("p (l s) -> p l s", l=NL)[:, :, 0:NS],
                BT[qi * 5:qi * 5 + 5, :])
        rowip = sml.tile([1, SL2], F32, name="rowiP")
        nc.gpsimd.memset(rowip[:], 0.0)
        nc.sync.dma_start(rowip[:].rearrange("p (l s) -> p l s", l=NL)[:, :, 0:NS],
                          BT[25:30, :])
        bciP = med.tile([128, SL2], F32)
        nc.gpsimd.partition_broadcast(bciP[:], rowip[:], channels=128)

        # ================= roots [128, SL2] =================
        psarg = ps1.tile([128, SL2], F32, tag="psarg")
        nc.tensor.matmul(psarg[:], Y3[:], Hrow3[:], start=True, stop=True)
        valid = med.tile([128, SL2], F32)
        nc.vector.tensor_scalar(valid[:], psarg[:], 0.0, None, Alu.is_ge)
        arg = med.tile([128, SL2], F32)
        nc.vector.tensor_scalar(arg[:], psarg[:], 0.0, None, Alu.max)
        rt = med.tile([128, SL2], F32)
        nc.scalar.sqrt(rt[:], arg[:])
        rrec = med.tile([128, SL2], F32, tag="tq2")
        nc.vector.tensor_scalar(rrec[:], rt[:], 1e-30, None, Alu.max)
        nc.vector.reciprocal(rrec[:], rrec[:])
        nc.vector.tensor_tensor(out=rrec[:], in0=arg[:], in1=rrec[:], op=Alu.mult)
        nc.vector.tensor_tensor(out=rt[:], in0=rt[:], in1=rrec[:], op=Alu.add)
        nc.vector.tensor_scalar(rt[:], rt[:], 0.5, None, Alu.mult)
        half = rt
        nc.vector.tensor_tensor(out=half[:], in0=rt[:], in1=bciP[:], op=Alu.mult)
        psmrow = ps1.tile([128, SL2], F32, tag="psarg")
        nc.tensor.matmul(psmrow[:], Y2[:], Mrow2[:], start=True, stop=True)
        xlo = med.tile([128, SL2], F32)
        nc.vector.tensor_tensor(out=xlo[:], in0=psmrow[:], in1=half[:], op=Alu.subtract)
        nc.vector.tensor_scalar(xlo[:], xlo[:], 0.0, 127.0, Alu.max, Alu.min)
        xhi = med.tile([128, SL2], F32)
        nc.vector.tensor_tensor(out=xhi[:], in0=psmrow[:], in1=half[:], op=Alu.add)
        nc.vector.tensor_scalar(xhi[:], xhi[:], 0.0, 127.0, Alu.max, Alu.min)
        # nhi = floor(xhi), nlo = ceil(xlo) via int truncation
        nint = med.tile([128, SL2], I32, tag="nint")
        nc.vector.tensor_copy(nint[:], xhi[:])
        nhi = med.tile([128, SL2], F32)
        nc.vector.tensor_copy(nhi[:], nint[:])
        fhi = med.tile([128, SL2], F32, tag="tq3")
        nc.vector.tensor_tensor(out=fhi[:], in0=nhi[:], in1=xhi[:], op=Alu.is_gt)
        nc.vector.tensor_tensor(out=nhi[:], in0=nhi[:], in1=fhi[:], op=Alu.subtract)
        nc.vector.tensor_copy(nint[:], xlo[:])
        nlo = med.tile([128, SL2], F32)
        nc.vector.tensor_copy(nlo[:], nint[:])
        frac = med.tile([128, SL2], F32, tag="tq3")
        nc.vector.tensor_tensor(out=frac[:], in0=xlo[:], in1=nlo[:], op=Alu.is_gt)
        nc.vector.tensor_tensor(out=nlo[:], in0=nlo[:], in1=frac[:], op=Alu.add)
        cnt = med.tile([128, SL2], F32)
        nc.vector.tensor_tensor(out=cnt[:], in0=nhi[:], in1=nlo[:], op=Alu.subtract)
        nc.vector.tensor_scalar(cnt[:], cnt[:], 1.0, 0.0, Alu.add, Alu.max)
        nc.vector.tensor_tensor(out=cnt[:], in0=cnt[:], in1=valid[:], op=Alu.mult)
        okm = med.tile([128, SL2], F32, tag="tq")
        nc.vector.tensor_scalar(okm[:], cnt[:], 0.5, None, Alu.is_ge)

        # markers: idx = ok ? (n + 130*sl) : -1, interleaved (lo, hi) pairs
        M16 = med.tile([128, SL2 * 2], I16)
        m16v = M16[:].rearrange("p (q two) -> p q two", two=2)
        tmod = med.tile([128, SL2], F32, tag="tq2")
        nc.vector.tensor_tensor(out=tmod[:], in0=nlo[:], in1=LB1[:], op=Alu.add)
        nc.vector.tensor_tensor(out=tmod[:], in0=tmod[:], in1=okm[:], op=Alu.mult)
        nc.vector.tensor_scalar(tmod[:], tmod[:], 1.0, None, Alu.subtract)
        nc.vector.tensor_copy(m16v[:, :, 0], tmod[:])
        nc.vector.tensor_scalar(nhi[:], nhi[:], 1.0, None, Alu.add)
        nc.vector.tensor_tensor(out=tmod[:], in0=nhi[:], in1=LB1[:], op=Alu.add)
        nc.vector.tensor_tensor(out=tmod[:], in0=tmod[:], in1=okm[:], op=Alu.mult)
        nc.vector.tensor_scalar(tmod[:], tmod[:], 1.0, None, Alu.subtract)
        nc.vector.tensor_copy(m16v[:, :, 1], tmod[:])

        # ======== speculative output fields/moments/fit (all 5 levels) ========
        # von (centered-normalized vo, in place) makes the output-side field
        # thresholds level-constants, so fields + moments + fit for every
        # candidate level run during the scatter/interval phase; the argmax
        # then just selects.
        SB = med.tile([NS, 60], F32)
        for l in range(NL):
            fon = fg.tile([128, F], BF16, name=f"fon{l}", tag="ffield")
            nc.scalar.activation(fon[:], vo[:, 0:F], Act.Relu, bias=lvl_bias[l][:], scale=1.0)
            gon = fg.tile([128, F], BF16, name=f"gon{l}", tag="gfield")
            nc.scalar.activation(gon[:], vo[:, 0:F], Act.Sign, bias=lvl_bias[l][:], scale=1.0)
            psf = moments_mm(fon, 20 + l)
            psg = moments_mm(gon, 25 + l)
            moments_fold(psf, 20 + l, SB, l * 6)
            moments_fold(psg, 25 + l, SB, 30 + l * 6)
        WallOn = med.tile([NS, 30], F32, name="WallOn")
        nc.vector.tensor_tensor(out=WallOn[:], in0=SB[:, 30:60], in1=cbt[:], op=Alu.add)
        nc.vector.tensor_tensor(out=WallOn[:], in0=WallOn[:], in1=hlvl[:], op=Alu.mult)
        nc.vector.tensor_tensor(out=WallOn[:], in0=WallOn[:], in1=SB[:, 0:30], op=Alu.add)
        WmO = {j: WallOn[:, j:30:6] for j in range(6)}
        po = fit_params(WmO, "po", NL, False, refine=False)

        # ================= marker scatter + I1 =================
        # scatter +-1 markers, multiply by P2 prefix in place, fold segment
        # halves (130 -> 65) in bf16, then reduce the folded half-segments.
        red = med.tile([128, SL2], F32, name="red")
        fold = big.tile([128, NSLOT * 65], BF16, name="fold", tag="bigA")
        for l in range(NL):
            D = dpool.tile([128, DW], BF16, name=f"D{l}", tag="dtile")
            for w in range(NW):
                base = (l * NW + w) * (SPW * 2)
                nc.gpsimd.local_scatter(
                    D[:, w * WELEM:(w + 1) * WELEM],
                    data26[:],
                    M16[:, base:base + SPW * 2],
                    channels=128, num_elems=WELEM, num_idxs=SPW * 2)
            nc.vector.tensor_tensor(out=D[:], in0=D[:], in1=P2[:], op=Alu.mult)
            Dv = D[:].rearrange("p (s c) -> p s c", s=NSLOT)
            fv65 = fold[:].rearrange("p (s c) -> p s c", s=NSLOT)
            nc.vector.tensor_tensor(out=fv65, in0=Dv[:, :, 0:65],
                                    in1=Dv[:, :, 65:130], op=Alu.add)
            nc.vector.tensor_reduce(
                red[:, l * NSLOT:(l + 1) * NSLOT], fv65, AX.X, Alu.add)

        psI1 = ps1.tile([1, SL2], F32, tag="psI1")
        nc.tensor.matmul(psI1[:], onescol[:], red[:], start=True, stop=True)
        psI0 = ps1.tile([1, SL2], F32, tag="psb")
        nc.tensor.matmul(psI0[:], onescol[:], cnt[:], start=True, stop=True)

        # ================= metric + argmax (on [1, SL] (l,64) layout) ==============
        SL = NL * NS
        I1r = sml.tile([1, SL], F32)
        nc.vector.tensor_copy(I1r[:].rearrange("p (l s) -> p l s", l=NL),
                              psI1[:].rearrange("p (l s) -> p l s", l=NL)[:, :, 0:NS])
        I0r = sml.tile([1, SL], F32)
        nc.vector.tensor_copy(I0r[:].rearrange("p (l s) -> p l s", l=NL),
                              psI0[:].rearrange("p (l s) -> p l s", l=NL)[:, :, 0:NS])
        iin = sml.tile([1, SL], F32)
        nc.vector.tensor_scalar(iin[:], I0r[:], 0.5, None, Alu.mult)
        nc.vector.tensor_tensor(out=iin[:], in0=iin[:], in1=I1r[:], op=Alu.add)
        met = sml.tile([1, SL], F32)
        nc.vector.tensor_tensor(out=met[:].rearrange("p (l s) -> p s l", l=NL),
                                in0=iin[:].rearrange("p (l s) -> p s l", l=NL),
                                in1=itr[:].to_broadcast((1, NS, NL)), op=Alu.mult)
        nc.vector.tensor_scalar(I0r[:], I0r[:], float(1.0 / NPIX), None, Alu.mult)
        nc.vector.tensor_tensor(out=met[:], in0=met[:], in1=I0r[:], op=Alu.subtract)
        mmax = sml.tile([1, NS], F32)
        nc.vector.tensor_reduce(mmax[:], met[:].rearrange("p (l s) -> p s l", l=NL), AX.X, Alu.max)
        lidx_i = sml.tile([1, SL], I32)
        nc.gpsimd.iota(lidx_i[:].rearrange("p (l s) -> p l s", l=NL),
                       pattern=[[1, NL], [0, NS]], base=0, channel_multiplier=0)
        cand = sml.tile([1, SL], F32)
        nc.vector.tensor_copy(cand[:], lidx_i[:])
        eqmax = sml.tile([1, SL], F32)
        nc.vector.tensor_tensor(out=eqmax[:].rearrange("p (l s) -> p s l", l=NL),
                                in0=met[:].rearrange("p (l s) -> p s l", l=NL),
                                in1=mmax[:].to_broadcast((1, NS, NL)), op=Alu.is_lt)
        nc.vector.tensor_scalar(eqmax[:], eqmax[:], 99.0, None, Alu.mult)
        nc.vector.tensor_tensor(out=cand[:], in0=cand[:], in1=eqmax[:], op=Alu.add)
        bestr = sml.tile([1, NS], F32)
        nc.vector.tensor_reduce(bestr[:], cand[:].rearrange("p (l s) -> p s l", l=NL), AX.X, Alu.min)

        # ================= select best level + sym loss =================
        def row2col(rowap, nm):
            pr = ps1.tile([NS, 1], F32, name=f"pr{nm}", tag="psmisc")
            nc.tensor.transpose(pr[:], rowap, eye128[0:1, 0:1])
            c = med.tile([NS, 1], F32, name=f"col{nm}")
            nc.vector.tensor_copy(c[:], pr[:])
            return c

        bestc = row2col(bestr[:], "best")

        def col(nm):
            return med.tile([NS, 1], F32, name=nm)

        eqm = med.tile([NS, NL], F32, name="eqm")
        l5f = med.tile([NS, NL], F32, name="l5f")
        nc.vector.tensor_copy(l5f[:], lr_i[:])
        nc.vector.tensor_scalar(eqm[:], l5f[:], bestc[:], None, Alu.is_equal)

        def select(src, nm):
            o = med.tile([NS, 1], F32, name="sel" + nm)
            tmp = med.tile([NS, NL], F32, name="selt" + nm, tag="seltmp")
            nc.vector.tensor_tensor(out=tmp[:], in0=src[:], in1=eqm[:], op=Alu.mult)
            nc.vector.tensor_reduce(o[:], tmp[:], AX.X, Alu.add)
            return o

        cxT = select(pt["cx"], "cx"); cyT = select(pt["cy"], "cy")
        cthT = select(pt["cth"], "ct"); sthT = select(pt["sth"], "st")
        aT = select(pt["a"], "a"); bT = select(pt["b"], "b")
        cxO = select(po["cx"], "cxO"); cyO = select(po["cy"], "cyO")
        cthO = select(po["cth"], "ctO"); sthO = select(po["sth"], "stO")
        aO = select(po["a"], "aO"); bO = select(po["b"], "bO")

        sc = col("sc")
        nc.vector.tensor_tensor(out=sc[:], in0=aO[:], in1=bO[:], op=Alu.max)
        t1c = col("t1c")
        nc.vector.tensor_tensor(out=t1c[:], in0=aT[:], in1=bT[:], op=Alu.max)
        nc.vector.tensor_tensor(out=sc[:], in0=sc[:], in1=t1c[:], op=Alu.max)
        nc.vector.tensor_scalar(sc[:], sc[:], float(EPS), None, Alu.add)
        isc = col("isc")
        nc.vector.reciprocal(isc[:], sc[:])
        lossc = col("lossc")
        td = col("td")

        def sqdiff_acc(xo, xt, first=False):
            nc.vector.tensor_tensor(out=td[:], in0=xo, in1=xt, op=Alu.subtract)
            nc.vector.tensor_tensor(out=td[:], in0=td[:], in1=isc[:], op=Alu.mult)
            nc.vector.tensor_tensor(out=td[:], in0=td[:], in1=td[:], op=Alu.mult)
            if first:
                nc.vector.tensor_copy(lossc[:], td[:])
            else:
                nc.vector.tensor_tensor(out=lossc[:], in0=lossc[:], in1=td[:], op=Alu.add)

        sqdiff_acc(cxO[:], cxT[:], first=True)
        sqdiff_acc(cyO[:], cyT[:])
        sqdiff_acc(aO[:], aT[:])
        sqdiff_acc(bO[:], bT[:])
        nc.vector.tensor_scalar(lossc[:], lossc[:], 0.5, None, Alu.mult)
        csum = col("csum")
        nc.vector.tensor_tensor(out=csum[:], in0=cthO[:], in1=cthT[:], op=Alu.mult)
        nc.vector.tensor_tensor(out=td[:], in0=sthO[:], in1=sthT[:], op=Alu.mult)
        nc.vector.tensor_tensor(out=csum[:], in0=csum[:], in1=td[:], op=Alu.add)
        nc.vector.tensor_scalar(csum[:], csum[:], -1.0, 1.0, Alu.mult, Alu.add)
        nc.vector.tensor_tensor(out=lossc[:], in0=lossc[:], in1=csum[:], op=Alu.add)

        nc.sync.dma_start(loss_out[:, :], lossc[:])


def build(NS=64, num_devices=1):
    nc = bacc.Bacc("TRN2", target_bir_lowering=False, debug=False, num_devices=num_devices)
    with tile.TileContext(nc) as tc:
        emit(nc, tc, NS=NS)
    nc.compile()
    return nc


_CACHED = {}


def _get_nc():
    if "nc" not in _CACHED:
        _CACHED["nc"] = build(NS=64, num_devices=8)
    return _CACHED["nc"]


def _make_in_maps(output, target, n_cores=8):
    output = np.ascontiguousarray(output, dtype=np.float32)
    target = np.ascontiguousarray(target, dtype=np.float32)
    per = output.shape[0] // n_cores
    in_maps = []
    for c in range(n_cores):
        sl = slice(c * per, (c + 1) * per)
        in_maps.append({
            "t": np.ascontiguousarray(target[sl].transpose(1, 0, 2)),
            "o": np.ascontiguousarray(output[sl].transpose(1, 0, 2)),
        })
    return in_maps


def kernel(output, target):
    from concourse.bass_utils import run_bass_kernel_spmd

    nc = _get_nc()
    in_maps = _make_in_maps(output, target)
    res = run_bass_kernel_spmd(nc, in_maps, core_ids=list(range(8)))
    losses = np.concatenate([r["loss"].reshape(-1) for r in res.results])
    return np.float32(losses.mean(dtype=np.float64))
